# revision 26
# baseline (speedup 1.0000x reference)
"""Trainium2 Bass kernel for nn_Net_21947282882692 (segment_reduce).

Strategy (8 NeuronCores, SPMD):
  - Data-parallel over the T*B*A segment axis == T axis (each core owns 8 of
    64 timesteps => 65536 of 524288 actions_table rows, 16384 segments).
  - The tiny MLP/LSTM params are replicated; every core runs the full
    T=64-step LSTM scan (cheap, latency-bound) interleaved with streaming
    its actions_table shard (DMA/PE-bound) through the shared t_fc tower and
    the offset-based segment sum (offset==4 => sum of 4 consecutive rows).
  - The final pointwise MLP + argmax + baseline run on each core's shard.
  - Host gathers shards along T.
"""

import sys

for _p in ("/opt/pypackages", "/opt/trn_rl_repo"):
    if _p not in sys.path:
        sys.path.insert(0, _p)

from contextlib import ExitStack

import numpy as np
import concourse.bass as bass
import concourse.mybir as mybir
from concourse import bacc
from concourse.tile import TileContext
from concourse.bass_utils import run_bass_kernel_spmd

F32 = mybir.dt.float32
F32R = mybir.dt.float32r
BF16 = mybir.dt.bfloat16
I32 = mybir.dt.int32
AF = mybir.ActivationFunctionType
ALU = mybir.AluOpType
AX = mybir.AxisListType

T, B, A = 64, 32, 64
TD, SD, AD = 128, 64, 16
K = 4
S = T * B * A            # 131072 segments
N = S * K                # 524288 ragged rows
H = 64
NCORES = 8
Tc = T // NCORES         # 8 timesteps per core
TBc = Tc * B             # 256 (t,b) rows per core
Sc = S // NCORES         # 16384 segments per core
Rc = N // NCORES         # 65536 actions_table rows per core

CHUNK = 2048             # actions rows per stream chunk (one per 2 steps)
SEGC = CHUNK // K        # 512 segments per chunk
NCHUNK = Rc // CHUNK     # 32
MCH = 512                # MLP chunk (segments)
NMCH = Sc // MCH         # 32


def _build_program(mask_steps):
    nc = bacc.Bacc("TRN2", target_bir_lowering=False, debug=False,
                   num_devices=NCORES)

    def din(name, shape, dt=F32):
        return nc.dram_tensor(name, shape, dt, kind="ExternalInput")

    def dout(name, shape, dt=F32):
        return nc.dram_tensor(name, shape, dt, kind="ExternalOutput")

    atT = din("atT", [TD, Rc])
    aoT = din("aoT", [AD, Sc])
    stT = din("stT", [TD, T * B])
    soT = din("soT", [SD, T * B])
    h0T = din("h0T", [2 * H, B])
    c0T = din("c0T", [2 * H, B])
    ndT = din("ndT", [128, T * B])
    Wt0T = din("Wt0T", [TD, 128]); bt0 = din("bt0", [128, 1])
    Wt1T = din("Wt1T", [128, 32]); bt1r4 = din("bt1r4", [128, 1])
    Ws0T = din("Ws0T", [SD, 32]); bs0 = din("bs0", [32, 1])
    Wa0T = din("Wa0T", [AD, 32]); ba0 = din("ba0", [32, 1])
    Wl = {}
    for l in (0, 1):
        for ch in ("a", "b"):
            Wl[l, ch] = din(f"Wc{l}{ch}", [128, 128])
        Wl[l, "ba"] = din(f"bls{l}a", [128, 1])
        Wl[l, "bb"] = din(f"bls{l}b", [128, 1])
    Wp0aT = din("Wp0aT", [H, 128])
    Wp0sT = din("Wp0sT", [H, 128])
    Wp0pT = din("Wp0pT", [H, 128])
    bp0 = din("bp0", [128, 1])
    Wp1T = din("Wp1T", [128, 128]); bp1 = din("bp1", [128, 1])
    Wp2T = din("Wp2T", [128, 64]); bp2 = din("bp2", [64, 1])
    wp3T = din("wp3T", [64, 1]); bp3 = din("bp3", [1, 1])
    Wb0T = din("Wb0T", [H, 64]); bb0 = din("bb0", [64, 1])
    wb1T = din("wb1T", [64, 1]); bb1 = din("bb1", [1, 1])
    i4 = din("i4", [128, 32])
    iota = din("iotaA", [64, 4 * A])

    out_logits = dout("out_logits", [Sc])
    out_action = dout("out_action", [TBc], I32)
    out_baseline = dout("out_baseline", [TBc])
    out_h = dout("out_h", [2, B, H])
    out_c = dout("out_c", [2, B, H])
    state_scr = nc.dram_tensor("state_scr", [H, T * B], F32R)

    with TileContext(nc) as tc:
        with tc.tile_pool(name="consts", bufs=1) as cpool, \
             tc.tile_pool(name="big", bufs=1) as big:

            _dmaq = [nc.sync, nc.scalar]
            _dmaqi = [0]

            def ctile(dram, shape, dt=F32):
                t = cpool.tile(shape, dt, tag=dram.name)
                src = dram[:] if dt != F32R else dram[:].bitcast(F32R)
                eng = _dmaq[_dmaqi[0] % 2]
                _dmaqi[0] += 1
                eng.dma_start(t[:], src)
                return t

            wt0 = ctile(Wt0T, [TD, 128], F32R)
            wt1 = ctile(Wt1T, [128, 32], F32R)
            ws0 = ctile(Ws0T, [SD, 32], F32R)
            wa0 = ctile(Wa0T, [AD, 32], F32R)
            bt0t = ctile(bt0, [128, 1]); bt1t = ctile(bt1r4, [128, 1])
            bs0t = ctile(bs0, [32, 1]); ba0t = ctile(ba0, [32, 1])
            wl = {}
            for l in (0, 1):
                for ch in ("a", "b"):
                    wl[l, ch] = ctile(Wl[l, ch], [128, 128])
                wl[l, "ba"] = ctile(Wl[l, "ba"], [128, 1])
                wl[l, "bb"] = ctile(Wl[l, "bb"], [128, 1])
            wp0a = ctile(Wp0aT, [H, 128], F32R)
            wp0s = ctile(Wp0sT, [H, 128], F32R)
            wp0p = ctile(Wp0pT, [H, 128], F32R)
            bp0t = ctile(bp0, [128, 1])
            wp1 = ctile(Wp1T, [128, 128], F32R); bp1t = ctile(bp1, [128, 1])
            wp2 = ctile(Wp2T, [128, 64], F32R); bp2t = ctile(bp2, [64, 1])
            wp3 = ctile(wp3T, [64, 1], F32R); bp3t = ctile(bp3, [1, 1])
            wb0 = ctile(Wb0T, [H, 64], F32R); bb0t = ctile(bb0, [64, 1])
            wb1 = ctile(wb1T, [64, 1], F32R); bb1t = ctile(bb1, [1, 1])
            i4t = ctile(i4, [128, 32], F32R)
            iot = ctile(iota, [64, 4 * A])
            ndt = ctile(ndT, [128, T * B]) if mask_steps else None

            sa64 = big.tile([H, Sc], F32R)

            # ---------------- state tower ----------------
            scan_ctx = ExitStack()
            scan_pool = scan_ctx.enter_context(
                tc.tile_pool(name="scan", bufs=1))
            X0 = scan_pool.tile([128, T * B + B], F32)
            X1 = scan_pool.tile([128, T * B + B], F32)
            with tc.tile_pool(name="st_ps", bufs=2, space="PSUM") as stps, \
                 tc.tile_pool(name="st_sb", bufs=1) as stsb:
                stx = stsb.tile([TD, T * B], F32R, tag="stx")
                nc.sync.dma_start(stx[:], stT[:].bitcast(F32R))
                sox = stsb.tile([SD, T * B], F32R, tag="sox")
                nc.sync.dma_start(sox[:], soT[:].bitcast(F32R))
                st1 = stsb.tile([128, T * B], F32R)
                for cidx in range(0, 4):
                    sl = bass.ts(cidx, 512)
                    p = stps.tile([128, 512], F32, tag="p")
                    nc.tensor.matmul(p[:], wt0[:], stx[:, sl], start=True,
                                     stop=True)
                    nc.scalar.activation(st1[:, sl], p[:], AF.Relu,
                                         bias=bt0t[:])
                for cidx in range(4):
                    sl = bass.ts(cidx, 512)
                    p2 = stps.tile([32, 512], F32, tag="p2")
                    nc.tensor.matmul(p2[:], wt1[:], st1[:, sl], start=True,
                                     stop=True)
                    nc.scalar.activation(X0[0:32, sl], p2[:], AF.Relu,
                                         bias=bt1t[0:32, :])
                    p3 = stps.tile([32, 512], F32, tag="p3")
                    nc.tensor.matmul(p3[:], ws0[:], sox[:, sl], start=True,
                                     stop=True)
                    nc.scalar.activation(X0[32:64, sl], p3[:], AF.Relu,
                                         bias=bs0t[:])

            # -------- interleaved LSTM scan + actions stream --------
            sctx = ExitStack()
            lstm_ps = sctx.enter_context(
                tc.tile_pool(name="lstm_ps", bufs=1, space="PSUM"))
            lstm_sb = sctx.enter_context(tc.tile_pool(name="lstm_sb", bufs=3))
            cst = sctx.enter_context(tc.tile_pool(name="cstate", bufs=2))
            xb_pool = sctx.enter_context(tc.tile_pool(name="xb", bufs=3))
            r1_pool = sctx.enter_context(tc.tile_pool(name="r1", bufs=2))
            f4_pool = sctx.enter_context(tc.tile_pool(name="f4", bufs=2))
            sps = sctx.enter_context(
                tc.tile_pool(name="sA", bufs=2, space="PSUM"))
            sps4 = sctx.enter_context(
                tc.tile_pool(name="s4", bufs=1, space="PSUM"))
            sps32 = sctx.enter_context(
                tc.tile_pool(name="s32", bufs=1, space="PSUM"))

            cprev = [None, None]
            nc.sync.dma_start(X0[64:128, 0:B], h0T[0:H, :])
            nc.sync.dma_start(X1[64:128, 0:B], h0T[0:H, :])
            nc.sync.dma_start(X1[0:64, 0:B], h0T[H:2 * H, :])
            for l in (0, 1):
                ct = cst.tile([H, B], F32, tag=f"c{l}")
                nc.sync.dma_start(ct[:], c0T[l * H:(l + 1) * H, :])
                cprev[l] = ct[:]

            def lstm_cell(l, t):
                stk = X0 if l == 0 else X1
                if t in mask_steps and l == 0:
                    for tile_, lo, hi in ((X0, 64, 128), (X1, 0, 64),
                                          (X1, 64, 128)):
                        nc.vector.tensor_tensor(
                            tile_[lo:hi, bass.ts(t, B)],
                            tile_[lo:hi, bass.ts(t, B)],
                            ndt[lo:hi, bass.ts(t, B)], op=ALU.mult)
                if t in mask_steps:
                    cm = cst.tile([H, B], F32, tag=f"cm{l}")
                    nc.vector.tensor_tensor(cm[:], cprev[l],
                                            ndt[0:H, bass.ts(t, B)],
                                            op=ALU.mult)
                    cprev[l] = cm[:]
                cp = cprev[l]
                stk_ap = stk[:, bass.ts(t, B)]
                pg = lstm_ps.tile([128, 2 * B], F32, tag=f"pg{l}")
                pa, pb = pg[:, 0:B], pg[:, B:2 * B]
                nc.tensor.matmul(pa, wl[l, "a"][:], stk_ap,
                                 start=True, stop=True)
                nc.tensor.matmul(pb, wl[l, "b"][:], stk_ap,
                                 start=True, stop=True)
                # critical-path order: f first (feeds r_), then g, i, o
                sf = lstm_sb.tile([H, B], F32, tag=f"sf{l}")
                nc.scalar.activation(sf[:], pa[H:128], AF.Sigmoid,
                                     bias=wl[l, "ba"][H:128, :])
                tg = lstm_sb.tile([H, B], F32, tag=f"tg{l}")
                nc.scalar.activation(tg[:], pb[0:H], AF.Tanh,
                                     bias=wl[l, "bb"][0:H, :])
                si = lstm_sb.tile([H, B], F32, tag=f"si{l}")
                nc.scalar.activation(si[:], pa[0:H], AF.Sigmoid,
                                     bias=wl[l, "ba"][0:H, :])
                so_ = lstm_sb.tile([H, B], F32, tag=f"so{l}")
                nc.scalar.activation(so_[:], pb[H:128], AF.Sigmoid,
                                     bias=wl[l, "bb"][H:128, :])
                r_ = lstm_sb.tile([H, B], F32, tag=f"r{l}")
                nc.gpsimd.tensor_tensor(r_[:], sf[:], cp, op=ALU.mult)
                p_ = lstm_sb.tile([H, B], F32, tag=f"p{l}")
                nc.gpsimd.tensor_tensor(p_[:], si[:], tg[:], op=ALU.mult)
                cn = cst.tile([H, B], F32, tag=f"c{l}")
                nc.gpsimd.tensor_tensor(cn[:], r_[:], p_[:], op=ALU.add)
                th = lstm_sb.tile([H, B], F32, tag=f"th{l}")
                nc.scalar.activation(th[:], cn[:], AF.Tanh)
                if l == 0:
                    hdst = X0[64:128, bass.ts(t + 1, B)]
                    nc.gpsimd.tensor_tensor(hdst, so_[:], th[:], op=ALU.mult)
                    nc.gpsimd.tensor_copy(X1[64:128, bass.ts(t, B)], hdst)
                else:
                    hdst = X1[0:64, bass.ts(t + 1, B)]
                    nc.gpsimd.tensor_tensor(hdst, so_[:], th[:], op=ALU.mult)
                cprev[l] = cn[:]

            def stream_half(k, half):
                if half == 0:
                    xb = xb_pool.tile([TD, CHUNK], F32R, tag="xb")
                    nc.sync.dma_start(xb[:],
                                      atT[:, bass.ts(k, CHUNK)].bitcast(F32R))
                    r1 = r1_pool.tile([128, CHUNK], F32R, tag="r1")
                    stream_state[k] = (xb, r1)
                xb, r1 = stream_state[k]
                for h_ in (0, 1) if half == 0 else (2, 3):
                    sl = bass.ts(h_, 512)
                    p1 = sps.tile([128, 512], F32, tag="p1")
                    nc.tensor.matmul(p1[:], wt0[:], xb[:, sl], start=True,
                                     stop=True)
                    if h_ % 2 == 0:
                        nc.scalar.activation(r1[:, sl], p1[:],
                                             AF.Relu, bias=bt0t[:])
                    else:
                        nc.vector.tensor_scalar(r1[:, sl], p1[:],
                                                bt0t[:], 0.0, op0=ALU.add,
                                                op1=ALU.max)
                if half == 0:
                    return
                r1v = r1[:].rearrange("p (s k) -> p s k", k=K)
                f4 = f4_pool.tile([128, SEGC], F32R, tag="f4")
                for b_ in range(K):
                    p4 = sps4.tile([32, SEGC], F32, tag=f"p4{b_ % 2}")
                    nc.tensor.matmul(p4[:], wt1[:], r1v[:, :, b_],
                                     start=True, stop=True)
                    dst = f4[bass.ts(b_, 32), :]
                    nc.vector.tensor_scalar(dst, p4[:], bt1t[0:32, :], 0.0,
                                            op0=ALU.add, op1=ALU.max)
                p32 = sps32.tile([32, SEGC], F32, tag="p32")
                nc.tensor.matmul(p32[:], i4t[:], f4[:], start=True, stop=True)
                nc.vector.tensor_scalar(sa64[0:32, bass.ts(k, SEGC)], p32[:],
                                        0.0, None, op0=ALU.add)

            ao_sb = sctx.enter_context(tc.tile_pool(name="ao_sb", bufs=3))

            def ao_chunk(k):
                sl = bass.ts(k, SEGC)
                aox = ao_sb.tile([AD, SEGC], F32R, tag="aox")
                nc.sync.dma_start(aox[:], aoT[:, sl].bitcast(F32R))
                p = sps32.tile([32, SEGC], F32, tag="pao")
                nc.tensor.matmul(p[:], wa0[:], aox[:], start=True, stop=True)
                if k % 2 == 0:
                    nc.scalar.activation(sa64[32:64, sl], p[:], AF.Relu,
                                         bias=ba0t[:])
                else:
                    nc.vector.tensor_scalar(sa64[32:64, sl], p[:], ba0t[:],
                                            0.0, op0=ALU.add, op1=ALU.max)

            stream_state = {}
            for t in range(T + 1):
                if t < T:
                    lstm_cell(0, t)
                if t > 0:
                    lstm_cell(1, t - 1)
                if t < T:
                    stream_half(t // 2, t % 2)
                    if t % 2 == 0:
                        ao_chunk(t // 2)

            hprev = [X0[64:128, bass.ts(T, B)], X1[0:64, bass.ts(T, B)]]
            # final h/c back to [B, H] layout and out.
            # blockwise DVE transpose keeps partition offsets aligned;
            # the DMA access pattern undoes the block structure.
            with tc.tile_pool(name="hc_out", bufs=1) as hc:
                for idx, (pair, dst) in enumerate(
                        ((hprev, out_h), (cprev, out_c))):
                    for l in (0, 1):
                        ap = pair[l]
                        tmp = hc.tile([H, B], F32, tag=f"nat{idx}{l}")
                        nc.vector.transpose(tmp[0:32, :], ap[0:32, :])
                        nc.vector.transpose(tmp[32:64, :], ap[32:64, :])
                        # tmp[32g+i, j] == h[feature 32g+j, batch i]
                        for g in range(2):
                            nc.sync.dma_start(
                                dst[l][:, bass.ts(g, 32)],
                                tmp[bass.ts(g, 32), :])

            sctx.close()

            # ---- state shard extraction (SPMD via partition id) ----
            nc.sync.dma_start(state_scr[:],
                              X1[0:64, B:B + T * B].bitcast(F32R))
            scan_ctx.close()
            with tc.tile_pool(name="shard_sb", bufs=1) as shsb:
                shard = shsb.tile([H, TBc], F32R)
                pid = nc.sync.partition_id()
                nc.sync.dma_start(shard[:],
                                  state_scr[:, bass.ds(pid * TBc, TBc)])

                # ---- final MLP over the shard ----
                with tc.tile_pool(name="mlp_sb", bufs=2) as msb, \
                     tc.tile_pool(name="m_ps", bufs=2, space="PSUM") as mps, \
                     tc.tile_pool(name="m_ps2", bufs=2, space="PSUM") as mps2, \
                     tc.tile_pool(name="m_psL", bufs=2, space="PSUM") as mpsL:
                    # state part of p0 via one tiny matmul, broadcast-added
                    pU = mpsL.tile([128, TBc], F32, tag="pL")
                    nc.tensor.matmul(pU[:], wp0s[:], shard[:], start=True,
                                     stop=True)
                    ust = msb.tile([128, TBc], F32, tag="ust")
                    nc.scalar.copy(ust[:], pU[:])
                    for m in range(NMCH):
                        sl = bass.ts(m, MCH)
                        ntb = MCH // A
                        shb = shard[:, bass.ts(m, ntb)].broadcast_to(
                            [H, ntb, A])
                        prod = msb.tile([H, MCH], F32R, tag="prod")
                        nc.gpsimd.tensor_tensor(
                            prod[:].rearrange("p (g a) -> p g a", a=A),
                            sa64[:, sl], shb, op=ALU.mult)
                        p0 = mps.tile([128, MCH], F32, tag="p0")
                        nc.tensor.matmul(p0[:], wp0a[:], sa64[:, sl],
                                         start=True, stop=False)
                        nc.tensor.matmul(p0[:], wp0p[:], prod[:], start=False,
                                         stop=True)
                        nc.vector.tensor_tensor(
                            p0[:].rearrange("p (g a) -> p g a", a=A), p0[:].rearrange("p (g a) -> p g a", a=A),
                            ust[:, bass.ts(m, ntb)].broadcast_to(
                                [128, ntb, A]), op=ALU.add)
                        x1 = msb.tile([128, MCH], F32R, tag="x1")
                        if m % 2 == 0:
                            nc.scalar.activation(x1[:], p0[:],
                                                 AF.Relu, bias=bp0t[:])
                        else:
                            nc.vector.tensor_scalar(x1[:], p0[:],
                                                    bp0t[:], 0.0, op0=ALU.add,
                                                    op1=ALU.max)
                        p1_ = mps.tile([128, MCH], F32, tag="p1")
                        nc.tensor.matmul(p1_[:], wp1[:], x1[:], start=True,
                                         stop=True)
                        x2 = msb.tile([128, MCH], F32R, tag="x2")
                        if m % 2 == 1:
                            nc.scalar.activation(x2[:], p1_[:],
                                                 AF.Relu, bias=bp1t[:])
                        else:
                            nc.vector.tensor_scalar(x2[:], p1_[:],
                                                    bp1t[:], 0.0, op0=ALU.add,
                                                    op1=ALU.max)
                        p2_ = mps2.tile([64, MCH], F32, tag="p2")
                        nc.tensor.matmul(p2_[:], wp2[:], x2[:], start=True,
                                         stop=True)
                        x3 = msb.tile([64, MCH], F32R, tag="x3")
                        if m % 2 == 0:
                            nc.scalar.activation(x3[:], p2_[:],
                                                 AF.Relu, bias=bp2t[:])
                        else:
                            nc.vector.tensor_scalar(x3[:], p2_[:],
                                                    bp2t[:], 0.0, op0=ALU.add,
                                                    op1=ALU.max)
                        pL = mpsL.tile([1, MCH], F32, tag="pL")
                        nc.tensor.matmul(pL[:], wp3[:], x3[:], start=True,
                                         stop=True)
                        lgc = msb.tile([1, MCH], F32, tag="lgc")
                        if m % 2 == 0:
                            nc.scalar.activation(lgc[:], pL[:],
                                                 AF.Identity, bias=bp3t[:])
                        else:
                            nc.vector.tensor_scalar(lgc[:], pL[:],
                                                    bp3t[:], None, op0=ALU.add)
                        nc.sync.dma_start(
                            out_logits[bass.ts(m, MCH)].rearrange(
                                "(a b) -> a b", a=1), lgc[:])

                # ---- baseline ----
                with tc.tile_pool(name="bl", bufs=1) as blsb, \
                     tc.tile_pool(name="bl_ps", bufs=1, space="PSUM") as blps:
                    pb_ = blps.tile([64, TBc], F32, tag="pb")
                    nc.tensor.matmul(pb_[:], wb0[:], shard[:], start=True,
                                     stop=True)
                    bl1 = blsb.tile([64, TBc], F32R)
                    nc.scalar.activation(bl1[:], pb_[:], AF.Relu,
                                         bias=bb0t[:])
                    pb2 = blps.tile([1, TBc], F32, tag="pb2")
                    nc.tensor.matmul(pb2[:], wb1[:], bl1[:], start=True,
                                     stop=True)
                    blo = blsb.tile([1, TBc], F32)
                    nc.scalar.activation(blo[:], pb2[:], AF.Identity,
                                         bias=bb1t[:])
                    nc.sync.dma_start(
                        out_baseline[:].rearrange("(a b) -> a b", a=1),
                        blo[0:1, :])

                # ---- argmax over A per (t,b) row ----
                with tc.tile_pool(name="am", bufs=1) as am:
                    lgT = am.tile([64, TBc], F32)
                    nc.sync.dma_start(
                        lgT[:], out_logits[:].rearrange("(p f) -> p f", p=64))
                    lgv = lgT[:].rearrange("p (g a) -> p g a", a=A)
                    mx = am.tile([64, 4], F32)
                    nc.vector.tensor_reduce(mx[:], lgv, axis=AX.X, op=ALU.max)
                    eq = am.tile([64, 4 * A], F32)
                    nc.vector.tensor_tensor(
                        eq[:].rearrange("p (g a) -> p g a", a=A), lgv,
                        mx[:].broadcast_to([64, 4, A]), op=ALU.is_ge)
                    pr = am.tile([64, 4 * A], F32)
                    nc.vector.tensor_tensor(pr[:], eq[:], iot[:], op=ALU.mult)
                    idxf = am.tile([64, 4], F32)
                    nc.vector.tensor_reduce(
                        idxf[:], pr[:].rearrange("p (g a) -> p g a", a=A),
                        axis=AX.X, op=ALU.add)
                    idxi = am.tile([64, 4], I32)
                    nc.vector.tensor_copy(idxi[:], idxf[:])
                    nc.sync.dma_start(
                        out_action[:].rearrange("(p f) -> p f", p=64), idxi[:])

    nc.finalize()
    return nc


_CACHE = {}


def _get_program(mask_steps):
    key = tuple(mask_steps)
    if key not in _CACHE:
        _CACHE[key] = _build_program(key)
    return _CACHE[key]


def kernel(**inputs):
    inp = {k: np.asarray(v) for k, v in inputs.items()}
    offset = inp["offset"]
    assert offset.sum() == N, "unsupported ragged layout"
    assert not np.any(offset != K), "general-offset path not implemented"
    done = inp["done"].astype(bool)
    mask_steps = tuple(int(t) for t in range(T) if done[t].any())
    nc = _get_program(mask_steps)

    f32 = np.float32
    at = np.ascontiguousarray(inp["actions_table"], dtype=f32)
    ao = np.ascontiguousarray(inp["actions_other"], dtype=f32).reshape(S, AD)
    stT = np.ascontiguousarray(inp["state_table"].reshape(T * B, TD).T,
                               dtype=f32)
    soT = np.ascontiguousarray(inp["state_other"].reshape(T * B, SD).T,
                               dtype=f32)
    h0T = np.ascontiguousarray(
        np.concatenate([inp["h0"][0].T, inp["h0"][1].T], axis=0), dtype=f32)
    c0T = np.ascontiguousarray(
        np.concatenate([inp["c0"][0].T, inp["c0"][1].T], axis=0), dtype=f32)
    nd = (1.0 - done.astype(f32)).reshape(1, T * B)
    ndT = np.ascontiguousarray(np.broadcast_to(nd, (128, T * B)), dtype=f32)

    def tp(x):
        return np.ascontiguousarray(np.asarray(x, dtype=f32).T)

    def col(x):
        return np.ascontiguousarray(np.asarray(x, dtype=f32).reshape(-1, 1))

    com = {
        "stT": stT, "soT": soT, "h0T": h0T, "c0T": c0T, "ndT": ndT,
        "Wt0T": tp(inp["W_t0"]), "bt0": col(inp["b_t0"]),
        "Wt1T": tp(inp["W_t1"]), "bt1r4": col(np.tile(inp["b_t1"], 4)),
        "Ws0T": tp(inp["W_s0"]), "bs0": col(inp["b_s0"]),
        "Wa0T": tp(inp["W_a0"]), "ba0": col(inp["b_a0"]),
        "Wp0aT": tp(inp["W_p0"][:, 64:128]),
        "Wp0sT": tp(inp["W_p0"][:, 0:64]),
        "Wp0pT": tp(inp["W_p0"][:, 128:192]),
        "bp0": col(inp["b_p0"]),
        "Wp1T": tp(inp["W_p1"]), "bp1": col(inp["b_p1"]),
        "Wp2T": tp(inp["W_p2"]), "bp2": col(inp["b_p2"]),
        "wp3T": tp(inp["W_p3"]), "bp3": col(inp["b_p3"]),
        "Wb0T": tp(inp["W_b0"]), "bb0": col(inp["b_b0"]),
        "wb1T": tp(inp["W_b1"]), "bb1": col(inp["b_b1"]),
        "i4": np.tile(np.eye(32, dtype=f32), (4, 1)),
        "iotaA": np.tile(np.arange(A, dtype=f32), (64, 4)),
    }
    for l in (0, 1):
        wih = inp[f"Wih{l}"].astype(f32)
        whh = inp[f"Whh{l}"].astype(f32)
        if l == 0:
            wcat = np.concatenate([wih, whh], axis=1)     # [256, 128]
        else:
            wcat = np.concatenate([whh, wih], axis=1)     # stack is [h1; h0]
        bls = (inp[f"bih{l}"] + inp[f"bhh{l}"]).astype(f32)
        com[f"Wc{l}a"] = tp(wcat[0:128, :])
        com[f"Wc{l}b"] = tp(wcat[128:256, :])
        com[f"bls{l}a"] = col(bls[0:128])
        com[f"bls{l}b"] = col(bls[128:256])

    in_maps = []
    for i in range(NCORES):
        m = dict(com)
        m["atT"] = np.ascontiguousarray(at[i * Rc:(i + 1) * Rc].T)
        m["aoT"] = np.ascontiguousarray(ao[i * Sc:(i + 1) * Sc].T)
        in_maps.append(m)

    res = run_bass_kernel_spmd(nc, in_maps, list(range(NCORES)), trace=False)
    r = res.results

    logits = np.concatenate([r[i]["out_logits"] for i in range(NCORES)])
    logits = logits.reshape(T, B, A)
    baseline = np.concatenate(
        [r[i]["out_baseline"] for i in range(NCORES)]).reshape(T, B)
    action = np.concatenate(
        [r[i]["out_action"] for i in range(NCORES)]).reshape(T, B)
    action = _refine_ties(inp, logits, action.astype(np.int32))
    hT = r[0]["out_h"]
    cT = r[0]["out_c"]
    return logits, baseline, action, hT, cT


def _refine_ties(inp, logits, action):
    """f32r matmuls carry ~1e-4 relative noise; rows whose top-2 logit gap is
    below that can argmax differently than fp32. Recompute just those rows
    in float64 on the host (a handful of rows, ~10 MFLOP each)."""
    srt = np.sort(logits, axis=-1)
    gap = srt[..., -1] - srt[..., -2]
    thr = 1e-2 * np.abs(logits).max()
    rows = np.argwhere(gap < thr)
    if rows.size == 0:
        return action
    f64 = np.float64
    W_t0 = inp["W_t0"].astype(f64); b_t0 = inp["b_t0"].astype(f64)
    W_t1 = inp["W_t1"].astype(f64); b_t1 = inp["b_t1"].astype(f64)

    # full LSTM state in f64 (cheap: ~4 MFLOP)
    def relu(x):
        return np.maximum(x, 0.0)

    st = relu(relu(inp["state_table"].reshape(T * B, TD).astype(f64)
                   @ W_t0.T + b_t0) @ W_t1.T + b_t1)
    so = relu(inp["state_other"].reshape(T * B, SD).astype(f64)
              @ inp["W_s0"].astype(f64).T + inp["b_s0"].astype(f64))
    core_in = np.concatenate([st, so], axis=-1).reshape(T, B, H)
    notdone = 1.0 - inp["done"].astype(f64)
    h = inp["h0"].astype(f64).copy(); c = inp["c0"].astype(f64).copy()
    Wih = [inp["Wih0"].astype(f64), inp["Wih1"].astype(f64)]
    Whh = [inp["Whh0"].astype(f64), inp["Whh1"].astype(f64)]
    bl_ = [(inp["bih0"] + inp["bhh0"]).astype(f64),
           (inp["bih1"] + inp["bhh1"]).astype(f64)]

    def sig(x):
        return 1.0 / (1.0 + np.exp(-x))

    states = np.zeros((T, B, H), f64)
    for t in range(T):
        h = h * notdone[t][None, :, None]
        c = c * notdone[t][None, :, None]
        x = core_in[t]
        for l in (0, 1):
            g = x @ Wih[l].T + h[l] @ Whh[l].T + bl_[l]
            i_, f_, g_, o_ = np.split(g, 4, axis=-1)
            c[l] = sig(f_) * c[l] + sig(i_) * np.tanh(g_)
            x = sig(o_) * np.tanh(c[l])
            h[l] = x
        states[t] = h[1]

    at_tab = inp["actions_table"].astype(f64)
    ao_all = inp["actions_other"].reshape(S, AD).astype(f64)
    W_a0 = inp["W_a0"].astype(f64); b_a0 = inp["b_a0"].astype(f64)
    W_p0 = inp["W_p0"].astype(f64); b_p0 = inp["b_p0"].astype(f64)
    W_p1 = inp["W_p1"].astype(f64); b_p1 = inp["b_p1"].astype(f64)
    W_p2 = inp["W_p2"].astype(f64); b_p2 = inp["b_p2"].astype(f64)
    W_p3 = inp["W_p3"].astype(f64); b_p3 = inp["b_p3"].astype(f64)
    offs = inp["offset"].reshape(-1)
    cum = np.zeros(S + 1, np.int64)
    np.cumsum(offs, out=cum[1:])
    for t_, b_ in rows:
        tb = t_ * B + b_
        segs = tb * A + np.arange(A)
        at_rows = []
        for s_ in segs:
            rws = at_tab[cum[s_]:cum[s_ + 1]]
            ft = relu(relu(rws @ W_t0.T + b_t0) @ W_t1.T + b_t1)
            at_rows.append(ft.sum(axis=0))
        at_ = np.stack(at_rows)
        ao_ = relu(ao_all[segs] @ W_a0.T + b_a0)
        acts = np.concatenate([at_, ao_], axis=-1)
        srep = np.broadcast_to(states[t_, b_], (A, H))
        x = np.concatenate([srep, acts, srep * acts], axis=-1)
        x = relu(x @ W_p0.T + b_p0)
        x = relu(x @ W_p1.T + b_p1)
        x = relu(x @ W_p2.T + b_p2)
        lg = x @ W_p3.T + b_p3
        action[t_, b_] = int(np.argmax(lg[:, 0]))
    return action


# revision 27
# speedup vs baseline: 1.1565x; 1.1565x over previous
"""Trainium2 Bass kernel for nn_Net_21947282882692 (segment_reduce).

Strategy (8 NeuronCores, SPMD):
  - Data-parallel over the T*B*A segment axis == T axis (each core owns 8 of
    64 timesteps => 65536 of 524288 actions_table rows, 16384 segments).
  - The tiny MLP/LSTM params are replicated; every core runs the full
    T=64-step LSTM scan (cheap, latency-bound) interleaved with streaming
    its actions_table shard (DMA/PE-bound) through the shared t_fc tower and
    the offset-based segment sum (offset==4 => sum of 4 consecutive rows).
  - The final pointwise MLP + argmax + baseline run on each core's shard.
  - Host gathers shards along T.
"""

import sys

for _p in ("/opt/pypackages", "/opt/trn_rl_repo"):
    if _p not in sys.path:
        sys.path.insert(0, _p)

from contextlib import ExitStack

import numpy as np
import concourse.bass as bass
import concourse.mybir as mybir
from concourse import bacc
from concourse.tile import TileContext
from concourse.bass_utils import run_bass_kernel_spmd

F32 = mybir.dt.float32
F32R = mybir.dt.float32r
BF16 = mybir.dt.bfloat16
I32 = mybir.dt.int32
AF = mybir.ActivationFunctionType
ALU = mybir.AluOpType
AX = mybir.AxisListType

T, B, A = 64, 32, 64
TD, SD, AD = 128, 64, 16
K = 4
S = T * B * A            # 131072 segments
N = S * K                # 524288 ragged rows
H = 64
NCORES = 8
Tc = T // NCORES         # 8 timesteps per core
TBc = Tc * B             # 256 (t,b) rows per core
Sc = S // NCORES         # 16384 segments per core
Rc = N // NCORES         # 65536 actions_table rows per core

CHUNK = 2048             # actions rows per stream chunk (one per 2 steps)
SEGC = CHUNK // K        # 512 segments per chunk
NCHUNK = Rc // CHUNK     # 32
MCH = 512                # MLP chunk (segments)
NMCH = Sc // MCH         # 32


def _build_program(mask_steps):
    nc = bacc.Bacc("TRN2", target_bir_lowering=False, debug=False,
                   num_devices=NCORES)

    def din(name, shape, dt=F32):
        return nc.dram_tensor(name, shape, dt, kind="ExternalInput")

    def dout(name, shape, dt=F32):
        return nc.dram_tensor(name, shape, dt, kind="ExternalOutput")

    atT = din("atT", [TD, Rc])
    aoT = din("aoT", [AD, Sc])
    stT = din("stT", [TD, T * B])
    soT = din("soT", [SD, T * B])
    h0T = din("h0T", [2 * H, B])
    c0T = din("c0T", [2 * H, B])
    ndT = din("ndT", [128, T * B])
    Wt0T = din("Wt0T", [TD, 128]); bt0 = din("bt0", [128, 1])
    Wt1T = din("Wt1T", [128, 32]); bt1r4 = din("bt1r4", [128, 1])
    Ws0T = din("Ws0T", [SD, 32]); bs0 = din("bs0", [32, 1])
    Wa0T = din("Wa0T", [AD, 32]); ba0 = din("ba0", [32, 1])
    Wl = {}
    for l in (0, 1):
        for ch in ("a", "b"):
            Wl[l, ch] = din(f"Wc{l}{ch}", [128, 128])
        Wl[l, "ba"] = din(f"bls{l}a", [128, 1])
        Wl[l, "bb"] = din(f"bls{l}b", [128, 1])
    Wp0aT = din("Wp0aT", [H, 128])
    Wp0sT = din("Wp0sT", [H, 128])
    Wp0pT = din("Wp0pT", [H, 128])
    bp0 = din("bp0", [128, 1])
    Wp1T = din("Wp1T", [128, 128]); bp1 = din("bp1", [128, 1])
    Wp2T = din("Wp2T", [128, 64]); bp2 = din("bp2", [64, 1])
    wp3T = din("wp3T", [64, 1]); bp3 = din("bp3", [1, 1])
    Wb0T = din("Wb0T", [H, 64]); bb0 = din("bb0", [64, 1])
    wb1T = din("wb1T", [64, 1]); bb1 = din("bb1", [1, 1])
    i4 = din("i4", [128, 32])
    iota = din("iotaA", [64, 4 * A])

    out_logits = dout("out_logits", [Sc])
    out_action = dout("out_action", [TBc], I32)
    out_baseline = dout("out_baseline", [TBc])
    out_h = dout("out_h", [2, B, H])
    out_c = dout("out_c", [2, B, H])
    state_scr = nc.dram_tensor("state_scr", [H, T * B], F32R)

    with TileContext(nc) as tc:
        with tc.tile_pool(name="consts", bufs=1) as cpool, \
             tc.tile_pool(name="big", bufs=1) as big:

            _dmaq = [nc.sync, nc.scalar]
            _dmaqi = [0]

            def ctile(dram, shape, dt=F32):
                t = cpool.tile(shape, dt, tag=dram.name)
                src = dram[:] if dt != F32R else dram[:].bitcast(F32R)
                eng = _dmaq[_dmaqi[0] % 2]
                _dmaqi[0] += 1
                eng.dma_start(t[:], src)
                return t

            wt0 = ctile(Wt0T, [TD, 128], F32R)
            wt1 = ctile(Wt1T, [128, 32], F32R)
            ws0 = ctile(Ws0T, [SD, 32], F32R)
            wa0 = ctile(Wa0T, [AD, 32], F32R)
            bt0t = ctile(bt0, [128, 1]); bt1t = ctile(bt1r4, [128, 1])
            bs0t = ctile(bs0, [32, 1]); ba0t = ctile(ba0, [32, 1])
            wl = {}
            for l in (0, 1):
                for ch in ("a", "b"):
                    wl[l, ch] = ctile(Wl[l, ch], [128, 128])
                wl[l, "ba"] = ctile(Wl[l, "ba"], [128, 1])
                wl[l, "bb"] = ctile(Wl[l, "bb"], [128, 1])
            wp0a = ctile(Wp0aT, [H, 128], F32R)
            wp0s = ctile(Wp0sT, [H, 128], F32R)
            wp0p = ctile(Wp0pT, [H, 128], F32R)
            bp0t = ctile(bp0, [128, 1])
            wp1 = ctile(Wp1T, [128, 128], F32R); bp1t = ctile(bp1, [128, 1])
            wp2 = ctile(Wp2T, [128, 64], F32R); bp2t = ctile(bp2, [64, 1])
            wp3 = ctile(wp3T, [64, 1], F32R); bp3t = ctile(bp3, [1, 1])
            wb0 = ctile(Wb0T, [H, 64], F32R); bb0t = ctile(bb0, [64, 1])
            wb1 = ctile(wb1T, [64, 1], F32R); bb1t = ctile(bb1, [1, 1])
            i4t = ctile(i4, [128, 32], F32R)
            iot = ctile(iota, [64, 4 * A])
            ndt = ctile(ndT, [128, T * B]) if mask_steps else None

            sa64 = big.tile([H, Sc], F32R)

            # ---------------- state tower ----------------
            scan_ctx = ExitStack()
            scan_pool = scan_ctx.enter_context(
                tc.tile_pool(name="scan", bufs=1))
            X0 = scan_pool.tile([128, T * B + B], F32)
            X1 = scan_pool.tile([128, T * B + B], F32)
            with tc.tile_pool(name="st_ps", bufs=2, space="PSUM") as stps, \
                 tc.tile_pool(name="st_sb", bufs=1) as stsb:
                stx = stsb.tile([TD, T * B], F32R, tag="stx")
                nc.sync.dma_start(stx[:], stT[:].bitcast(F32R))
                sox = stsb.tile([SD, T * B], F32R, tag="sox")
                nc.sync.dma_start(sox[:], soT[:].bitcast(F32R))
                st1 = stsb.tile([128, T * B], F32R)
                # HAM warmup: sustained PE burst so the clock gate opens
                # before the scan (redundant rewrites of the first slice).
                pw = stps.tile([128, 512], F32, tag="p")
                for _ in range(10):
                    nc.tensor.matmul(pw[:], wt0[:], stx[:, 0:512], start=True,
                                     stop=True)
                nc.scalar.activation(st1[:, 0:512], pw[:], AF.Relu,
                                     bias=bt0t[:])
                for cidx in range(1, 4):
                    sl = bass.ts(cidx, 512)
                    p = stps.tile([128, 512], F32, tag="p")
                    nc.tensor.matmul(p[:], wt0[:], stx[:, sl], start=True,
                                     stop=True)
                    nc.scalar.activation(st1[:, sl], p[:], AF.Relu,
                                         bias=bt0t[:])
                for cidx in range(4):
                    sl = bass.ts(cidx, 512)
                    p2 = stps.tile([32, 512], F32, tag="p2")
                    nc.tensor.matmul(p2[:], wt1[:], st1[:, sl], start=True,
                                     stop=True)
                    nc.scalar.activation(X0[0:32, sl], p2[:], AF.Relu,
                                         bias=bt1t[0:32, :])
                    p3 = stps.tile([32, 512], F32, tag="p3")
                    nc.tensor.matmul(p3[:], ws0[:], sox[:, sl], start=True,
                                     stop=True)
                    nc.scalar.activation(X0[32:64, sl], p3[:], AF.Relu,
                                         bias=bs0t[:])

            # -------- interleaved LSTM scan + actions stream --------
            sctx = ExitStack()
            lstm_ps = sctx.enter_context(
                tc.tile_pool(name="lstm_ps", bufs=1, space="PSUM"))
            lstm_sb = sctx.enter_context(tc.tile_pool(name="lstm_sb", bufs=3))
            cst = sctx.enter_context(tc.tile_pool(name="cstate", bufs=2))
            xb_pool = sctx.enter_context(tc.tile_pool(name="xb", bufs=3))
            r1_pool = sctx.enter_context(tc.tile_pool(name="r1", bufs=2))
            f4_pool = sctx.enter_context(tc.tile_pool(name="f4", bufs=2))
            sps = sctx.enter_context(
                tc.tile_pool(name="sA", bufs=2, space="PSUM"))
            sps4 = sctx.enter_context(
                tc.tile_pool(name="s4", bufs=1, space="PSUM"))
            sps32 = sctx.enter_context(
                tc.tile_pool(name="s32", bufs=1, space="PSUM"))

            cprev = [None, None]
            nc.sync.dma_start(X0[64:128, 0:B], h0T[0:H, :])
            nc.sync.dma_start(X1[64:128, 0:B], h0T[0:H, :])
            nc.sync.dma_start(X1[0:64, 0:B], h0T[H:2 * H, :])
            for l in (0, 1):
                ct = cst.tile([H, B], F32, tag=f"c{l}")
                nc.sync.dma_start(ct[:], c0T[l * H:(l + 1) * H, :])
                cprev[l] = ct[:]

            def lstm_cell(l, t):
                stk = X0 if l == 0 else X1
                if t in mask_steps and l == 0:
                    for tile_, lo, hi in ((X0, 64, 128), (X1, 0, 64),
                                          (X1, 64, 128)):
                        nc.vector.tensor_tensor(
                            tile_[lo:hi, bass.ts(t, B)],
                            tile_[lo:hi, bass.ts(t, B)],
                            ndt[lo:hi, bass.ts(t, B)], op=ALU.mult)
                if t in mask_steps:
                    cm = cst.tile([H, B], F32, tag=f"cm{l}")
                    nc.vector.tensor_tensor(cm[:], cprev[l],
                                            ndt[0:H, bass.ts(t, B)],
                                            op=ALU.mult)
                    cprev[l] = cm[:]
                cp = cprev[l]
                stk_ap = stk[:, bass.ts(t, B)]
                pg = lstm_ps.tile([128, 2 * B], F32, tag=f"pg{l}")
                pa, pb = pg[:, 0:B], pg[:, B:2 * B]
                nc.tensor.matmul(pa, wl[l, "a"][:], stk_ap,
                                 start=True, stop=True)
                nc.tensor.matmul(pb, wl[l, "b"][:], stk_ap,
                                 start=True, stop=True)
                # critical-path order: f first (feeds r_), then g, i, o
                sf = lstm_sb.tile([H, B], F32, tag=f"sf{l}")
                nc.scalar.activation(sf[:], pa[H:128], AF.Sigmoid,
                                     bias=wl[l, "ba"][H:128, :])
                tg = lstm_sb.tile([H, B], F32, tag=f"tg{l}")
                nc.scalar.activation(tg[:], pb[0:H], AF.Tanh,
                                     bias=wl[l, "bb"][0:H, :])
                si = lstm_sb.tile([H, B], F32, tag=f"si{l}")
                nc.scalar.activation(si[:], pa[0:H], AF.Sigmoid,
                                     bias=wl[l, "ba"][0:H, :])
                so_ = lstm_sb.tile([H, B], F32, tag=f"so{l}")
                nc.scalar.activation(so_[:], pb[H:128], AF.Sigmoid,
                                     bias=wl[l, "bb"][H:128, :])
                r_ = lstm_sb.tile([H, B], F32, tag=f"r{l}")
                nc.gpsimd.tensor_tensor(r_[:], sf[:], cp, op=ALU.mult)
                p_ = lstm_sb.tile([H, B], F32, tag=f"p{l}")
                nc.gpsimd.tensor_tensor(p_[:], si[:], tg[:], op=ALU.mult)
                cn = cst.tile([H, B], F32, tag=f"c{l}")
                nc.gpsimd.tensor_tensor(cn[:], r_[:], p_[:], op=ALU.add)
                th = lstm_sb.tile([H, B], F32, tag=f"th{l}")
                nc.scalar.activation(th[:], cn[:], AF.Tanh)
                if l == 0:
                    hdst = X0[64:128, bass.ts(t + 1, B)]
                    nc.gpsimd.tensor_tensor(hdst, so_[:], th[:], op=ALU.mult)
                    nc.gpsimd.tensor_copy(X1[64:128, bass.ts(t, B)], hdst)
                else:
                    hdst = X1[0:64, bass.ts(t + 1, B)]
                    nc.gpsimd.tensor_tensor(hdst, so_[:], th[:], op=ALU.mult)
                cprev[l] = cn[:]

            def stream_half(k, half):
                if half == 0:
                    xb = xb_pool.tile([TD, CHUNK], F32R, tag="xb")
                    nc.sync.dma_start(xb[:],
                                      atT[:, bass.ts(k, CHUNK)].bitcast(F32R))
                    r1 = r1_pool.tile([128, CHUNK], F32R, tag="r1")
                    stream_state[k] = (xb, r1)
                xb, r1 = stream_state[k]
                for h_ in (0, 1) if half == 0 else (2, 3):
                    sl = bass.ts(h_, 512)
                    p1 = sps.tile([128, 512], F32, tag="p1")
                    nc.tensor.matmul(p1[:], wt0[:], xb[:, sl], start=True,
                                     stop=True)
                    if h_ % 2 == 0:
                        nc.scalar.activation(r1[:, sl], p1[:],
                                             AF.Relu, bias=bt0t[:])
                    else:
                        nc.vector.tensor_scalar(r1[:, sl], p1[:],
                                                bt0t[:], 0.0, op0=ALU.add,
                                                op1=ALU.max)
                if half == 0:
                    return
                r1v = r1[:].rearrange("p (s k) -> p s k", k=K)
                f4 = f4_pool.tile([128, SEGC], F32R, tag="f4")
                for b_ in range(K):
                    p4 = sps4.tile([32, SEGC], F32, tag=f"p4{b_ % 2}")
                    nc.tensor.matmul(p4[:], wt1[:], r1v[:, :, b_],
                                     start=True, stop=True)
                    dst = f4[bass.ts(b_, 32), :]
                    nc.vector.tensor_scalar(dst, p4[:], bt1t[0:32, :], 0.0,
                                            op0=ALU.add, op1=ALU.max)
                p32 = sps32.tile([32, SEGC], F32, tag="p32")
                nc.tensor.matmul(p32[:], i4t[:], f4[:], start=True, stop=True)
                nc.vector.tensor_scalar(sa64[0:32, bass.ts(k, SEGC)], p32[:],
                                        0.0, None, op0=ALU.add)

            ao_sb = sctx.enter_context(tc.tile_pool(name="ao_sb", bufs=3))

            def ao_chunk(k):
                sl = bass.ts(k, SEGC)
                aox = ao_sb.tile([AD, SEGC], F32R, tag="aox")
                nc.sync.dma_start(aox[:], aoT[:, sl].bitcast(F32R))
                p = sps32.tile([32, SEGC], F32, tag="pao")
                nc.tensor.matmul(p[:], wa0[:], aox[:], start=True, stop=True)
                if k % 2 == 0:
                    nc.scalar.activation(sa64[32:64, sl], p[:], AF.Relu,
                                         bias=ba0t[:])
                else:
                    nc.vector.tensor_scalar(sa64[32:64, sl], p[:], ba0t[:],
                                            0.0, op0=ALU.add, op1=ALU.max)

            stream_state = {}
            for t in range(T + 1):
                if t < T:
                    lstm_cell(0, t)
                if t > 0:
                    lstm_cell(1, t - 1)
                if t < T:
                    stream_half(t // 2, t % 2)
                    if t % 2 == 0:
                        ao_chunk(t // 2)

            hprev = [X0[64:128, bass.ts(T, B)], X1[0:64, bass.ts(T, B)]]
            # final h/c back to [B, H] layout and out.
            # blockwise DVE transpose keeps partition offsets aligned;
            # the DMA access pattern undoes the block structure.
            with tc.tile_pool(name="hc_out", bufs=1) as hc:
                for idx, (pair, dst) in enumerate(
                        ((hprev, out_h), (cprev, out_c))):
                    for l in (0, 1):
                        ap = pair[l]
                        tmp = hc.tile([H, B], F32, tag=f"nat{idx}{l}")
                        nc.vector.transpose(tmp[0:32, :], ap[0:32, :])
                        nc.vector.transpose(tmp[32:64, :], ap[32:64, :])
                        # tmp[32g+i, j] == h[feature 32g+j, batch i]
                        for g in range(2):
                            nc.sync.dma_start(
                                dst[l][:, bass.ts(g, 32)],
                                tmp[bass.ts(g, 32), :])

            sctx.close()

            # ---- state shard extraction (SPMD via partition id) ----
            nc.sync.dma_start(state_scr[:],
                              X1[0:64, B:B + T * B].bitcast(F32R))
            scan_ctx.close()
            with tc.tile_pool(name="shard_sb", bufs=1) as shsb:
                shard = shsb.tile([H, TBc], F32R)
                pid = nc.sync.partition_id()
                nc.sync.dma_start(shard[:],
                                  state_scr[:, bass.ds(pid * TBc, TBc)])

                # ---- final MLP over the shard ----
                with tc.tile_pool(name="mlp_sb", bufs=2) as msb, \
                     tc.tile_pool(name="m_ps", bufs=2, space="PSUM") as mps, \
                     tc.tile_pool(name="m_ps2", bufs=2, space="PSUM") as mps2, \
                     tc.tile_pool(name="m_psL", bufs=2, space="PSUM") as mpsL:
                    for m in range(NMCH):
                        sl = bass.ts(m, MCH)
                        ntb = MCH // A
                        shb = shard[:, bass.ts(m, ntb)].broadcast_to(
                            [H, ntb, A])
                        prod = msb.tile([H, MCH], F32R, tag="prod")
                        nc.gpsimd.tensor_tensor(
                            prod[:].rearrange("p (g a) -> p g a", a=A),
                            sa64[:, sl], shb, op=ALU.mult)
                        p0 = mps.tile([128, MCH], F32, tag="p0")
                        nc.tensor.matmul(p0[:], wp0a[:], sa64[:, sl],
                                         start=True, stop=False)
                        nc.tensor.matmul(p0[:], wp0p[:], prod[:], start=False,
                                         stop=False)
                        nc.tensor.matmul(p0[:], wp0s[:], shb, start=False,
                                         stop=True)
                        x1 = msb.tile([128, MCH], F32R, tag="x1")
                        if m % 2 == 0:
                            nc.scalar.activation(x1[:], p0[:],
                                                 AF.Relu, bias=bp0t[:])
                        else:
                            nc.vector.tensor_scalar(x1[:], p0[:],
                                                    bp0t[:], 0.0, op0=ALU.add,
                                                    op1=ALU.max)
                        p1_ = mps.tile([128, MCH], F32, tag="p1")
                        nc.tensor.matmul(p1_[:], wp1[:], x1[:], start=True,
                                         stop=True)
                        x2 = msb.tile([128, MCH], F32R, tag="x2")
                        if m % 2 == 1:
                            nc.scalar.activation(x2[:], p1_[:],
                                                 AF.Relu, bias=bp1t[:])
                        else:
                            nc.vector.tensor_scalar(x2[:], p1_[:],
                                                    bp1t[:], 0.0, op0=ALU.add,
                                                    op1=ALU.max)
                        p2_ = mps2.tile([64, MCH], F32, tag="p2")
                        nc.tensor.matmul(p2_[:], wp2[:], x2[:], start=True,
                                         stop=True)
                        x3 = msb.tile([64, MCH], F32R, tag="x3")
                        if m % 2 == 0:
                            nc.scalar.activation(x3[:], p2_[:],
                                                 AF.Relu, bias=bp2t[:])
                        else:
                            nc.vector.tensor_scalar(x3[:], p2_[:],
                                                    bp2t[:], 0.0, op0=ALU.add,
                                                    op1=ALU.max)
                        pL = mpsL.tile([1, MCH], F32, tag="pL")
                        nc.tensor.matmul(pL[:], wp3[:], x3[:], start=True,
                                         stop=True)
                        lgc = msb.tile([1, MCH], F32, tag="lgc")
                        if m % 2 == 0:
                            nc.scalar.activation(lgc[:], pL[:],
                                                 AF.Identity, bias=bp3t[:])
                        else:
                            nc.vector.tensor_scalar(lgc[:], pL[:],
                                                    bp3t[:], None, op0=ALU.add)
                        nc.sync.dma_start(
                            out_logits[bass.ts(m, MCH)].rearrange(
                                "(a b) -> a b", a=1), lgc[:])

                # ---- baseline ----
                with tc.tile_pool(name="bl", bufs=1) as blsb, \
                     tc.tile_pool(name="bl_ps", bufs=1, space="PSUM") as blps:
                    pb_ = blps.tile([64, TBc], F32, tag="pb")
                    nc.tensor.matmul(pb_[:], wb0[:], shard[:], start=True,
                                     stop=True)
                    bl1 = blsb.tile([64, TBc], F32R)
                    nc.scalar.activation(bl1[:], pb_[:], AF.Relu,
                                         bias=bb0t[:])
                    pb2 = blps.tile([1, TBc], F32, tag="pb2")
                    nc.tensor.matmul(pb2[:], wb1[:], bl1[:], start=True,
                                     stop=True)
                    blo = blsb.tile([1, TBc], F32)
                    nc.scalar.activation(blo[:], pb2[:], AF.Identity,
                                         bias=bb1t[:])
                    nc.sync.dma_start(
                        out_baseline[:].rearrange("(a b) -> a b", a=1),
                        blo[0:1, :])

                # ---- argmax over A per (t,b) row ----
                with tc.tile_pool(name="am", bufs=1) as am:
                    lgT = am.tile([64, TBc], F32)
                    nc.sync.dma_start(
                        lgT[:], out_logits[:].rearrange("(p f) -> p f", p=64))
                    lgv = lgT[:].rearrange("p (g a) -> p g a", a=A)
                    mx = am.tile([64, 4], F32)
                    nc.vector.tensor_reduce(mx[:], lgv, axis=AX.X, op=ALU.max)
                    eq = am.tile([64, 4 * A], F32)
                    nc.vector.tensor_tensor(
                        eq[:].rearrange("p (g a) -> p g a", a=A), lgv,
                        mx[:].broadcast_to([64, 4, A]), op=ALU.is_ge)
                    pr = am.tile([64, 4 * A], F32)
                    nc.vector.tensor_tensor(pr[:], eq[:], iot[:], op=ALU.mult)
                    idxf = am.tile([64, 4], F32)
                    nc.vector.tensor_reduce(
                        idxf[:], pr[:].rearrange("p (g a) -> p g a", a=A),
                        axis=AX.X, op=ALU.add)
                    idxi = am.tile([64, 4], I32)
                    nc.vector.tensor_copy(idxi[:], idxf[:])
                    nc.sync.dma_start(
                        out_action[:].rearrange("(p f) -> p f", p=64), idxi[:])

    nc.finalize()
    return nc


_CACHE = {}


def _get_program(mask_steps):
    key = tuple(mask_steps)
    if key not in _CACHE:
        _CACHE[key] = _build_program(key)
    return _CACHE[key]


def kernel(**inputs):
    inp = {k: np.asarray(v) for k, v in inputs.items()}
    offset = inp["offset"]
    assert offset.sum() == N, "unsupported ragged layout"
    assert not np.any(offset != K), "general-offset path not implemented"
    done = inp["done"].astype(bool)
    mask_steps = tuple(int(t) for t in range(T) if done[t].any())
    nc = _get_program(mask_steps)

    f32 = np.float32
    at = np.ascontiguousarray(inp["actions_table"], dtype=f32)
    ao = np.ascontiguousarray(inp["actions_other"], dtype=f32).reshape(S, AD)
    stT = np.ascontiguousarray(inp["state_table"].reshape(T * B, TD).T,
                               dtype=f32)
    soT = np.ascontiguousarray(inp["state_other"].reshape(T * B, SD).T,
                               dtype=f32)
    h0T = np.ascontiguousarray(
        np.concatenate([inp["h0"][0].T, inp["h0"][1].T], axis=0), dtype=f32)
    c0T = np.ascontiguousarray(
        np.concatenate([inp["c0"][0].T, inp["c0"][1].T], axis=0), dtype=f32)
    nd = (1.0 - done.astype(f32)).reshape(1, T * B)
    ndT = np.ascontiguousarray(np.broadcast_to(nd, (128, T * B)), dtype=f32)

    def tp(x):
        return np.ascontiguousarray(np.asarray(x, dtype=f32).T)

    def col(x):
        return np.ascontiguousarray(np.asarray(x, dtype=f32).reshape(-1, 1))

    com = {
        "stT": stT, "soT": soT, "h0T": h0T, "c0T": c0T, "ndT": ndT,
        "Wt0T": tp(inp["W_t0"]), "bt0": col(inp["b_t0"]),
        "Wt1T": tp(inp["W_t1"]), "bt1r4": col(np.tile(inp["b_t1"], 4)),
        "Ws0T": tp(inp["W_s0"]), "bs0": col(inp["b_s0"]),
        "Wa0T": tp(inp["W_a0"]), "ba0": col(inp["b_a0"]),
        "Wp0aT": tp(inp["W_p0"][:, 64:128]),
        "Wp0sT": tp(inp["W_p0"][:, 0:64]),
        "Wp0pT": tp(inp["W_p0"][:, 128:192]),
        "bp0": col(inp["b_p0"]),
        "Wp1T": tp(inp["W_p1"]), "bp1": col(inp["b_p1"]),
        "Wp2T": tp(inp["W_p2"]), "bp2": col(inp["b_p2"]),
        "wp3T": tp(inp["W_p3"]), "bp3": col(inp["b_p3"]),
        "Wb0T": tp(inp["W_b0"]), "bb0": col(inp["b_b0"]),
        "wb1T": tp(inp["W_b1"]), "bb1": col(inp["b_b1"]),
        "i4": np.tile(np.eye(32, dtype=f32), (4, 1)),
        "iotaA": np.tile(np.arange(A, dtype=f32), (64, 4)),
    }
    for l in (0, 1):
        wih = inp[f"Wih{l}"].astype(f32)
        whh = inp[f"Whh{l}"].astype(f32)
        if l == 0:
            wcat = np.concatenate([wih, whh], axis=1)     # [256, 128]
        else:
            wcat = np.concatenate([whh, wih], axis=1)     # stack is [h1; h0]
        bls = (inp[f"bih{l}"] + inp[f"bhh{l}"]).astype(f32)
        com[f"Wc{l}a"] = tp(wcat[0:128, :])
        com[f"Wc{l}b"] = tp(wcat[128:256, :])
        com[f"bls{l}a"] = col(bls[0:128])
        com[f"bls{l}b"] = col(bls[128:256])

    in_maps = []
    for i in range(NCORES):
        m = dict(com)
        m["atT"] = np.ascontiguousarray(at[i * Rc:(i + 1) * Rc].T)
        m["aoT"] = np.ascontiguousarray(ao[i * Sc:(i + 1) * Sc].T)
        in_maps.append(m)

    res = run_bass_kernel_spmd(nc, in_maps, list(range(NCORES)), trace=False)
    r = res.results

    logits = np.concatenate([r[i]["out_logits"] for i in range(NCORES)])
    logits = logits.reshape(T, B, A)
    baseline = np.concatenate(
        [r[i]["out_baseline"] for i in range(NCORES)]).reshape(T, B)
    action = np.concatenate(
        [r[i]["out_action"] for i in range(NCORES)]).reshape(T, B)
    action = _refine_ties(inp, logits, action.astype(np.int32))
    hT = r[0]["out_h"]
    cT = r[0]["out_c"]
    return logits, baseline, action, hT, cT


def _refine_ties(inp, logits, action):
    """f32r matmuls carry ~1e-4 relative noise; rows whose top-2 logit gap is
    below that can argmax differently than fp32. Recompute just those rows
    in float64 on the host (a handful of rows, ~10 MFLOP each)."""
    srt = np.sort(logits, axis=-1)
    gap = srt[..., -1] - srt[..., -2]
    thr = 1e-2 * np.abs(logits).max()
    rows = np.argwhere(gap < thr)
    if rows.size == 0:
        return action
    f64 = np.float64
    W_t0 = inp["W_t0"].astype(f64); b_t0 = inp["b_t0"].astype(f64)
    W_t1 = inp["W_t1"].astype(f64); b_t1 = inp["b_t1"].astype(f64)

    # full LSTM state in f64 (cheap: ~4 MFLOP)
    def relu(x):
        return np.maximum(x, 0.0)

    st = relu(relu(inp["state_table"].reshape(T * B, TD).astype(f64)
                   @ W_t0.T + b_t0) @ W_t1.T + b_t1)
    so = relu(inp["state_other"].reshape(T * B, SD).astype(f64)
              @ inp["W_s0"].astype(f64).T + inp["b_s0"].astype(f64))
    core_in = np.concatenate([st, so], axis=-1).reshape(T, B, H)
    notdone = 1.0 - inp["done"].astype(f64)
    h = inp["h0"].astype(f64).copy(); c = inp["c0"].astype(f64).copy()
    Wih = [inp["Wih0"].astype(f64), inp["Wih1"].astype(f64)]
    Whh = [inp["Whh0"].astype(f64), inp["Whh1"].astype(f64)]
    bl_ = [(inp["bih0"] + inp["bhh0"]).astype(f64),
           (inp["bih1"] + inp["bhh1"]).astype(f64)]

    def sig(x):
        return 1.0 / (1.0 + np.exp(-x))

    states = np.zeros((T, B, H), f64)
    for t in range(T):
        h = h * notdone[t][None, :, None]
        c = c * notdone[t][None, :, None]
        x = core_in[t]
        for l in (0, 1):
            g = x @ Wih[l].T + h[l] @ Whh[l].T + bl_[l]
            i_, f_, g_, o_ = np.split(g, 4, axis=-1)
            c[l] = sig(f_) * c[l] + sig(i_) * np.tanh(g_)
            x = sig(o_) * np.tanh(c[l])
            h[l] = x
        states[t] = h[1]

    at_tab = inp["actions_table"].astype(f64)
    ao_all = inp["actions_other"].reshape(S, AD).astype(f64)
    W_a0 = inp["W_a0"].astype(f64); b_a0 = inp["b_a0"].astype(f64)
    W_p0 = inp["W_p0"].astype(f64); b_p0 = inp["b_p0"].astype(f64)
    W_p1 = inp["W_p1"].astype(f64); b_p1 = inp["b_p1"].astype(f64)
    W_p2 = inp["W_p2"].astype(f64); b_p2 = inp["b_p2"].astype(f64)
    W_p3 = inp["W_p3"].astype(f64); b_p3 = inp["b_p3"].astype(f64)
    offs = inp["offset"].reshape(-1)
    cum = np.zeros(S + 1, np.int64)
    np.cumsum(offs, out=cum[1:])
    for t_, b_ in rows:
        tb = t_ * B + b_
        segs = tb * A + np.arange(A)
        at_rows = []
        for s_ in segs:
            rws = at_tab[cum[s_]:cum[s_ + 1]]
            ft = relu(relu(rws @ W_t0.T + b_t0) @ W_t1.T + b_t1)
            at_rows.append(ft.sum(axis=0))
        at_ = np.stack(at_rows)
        ao_ = relu(ao_all[segs] @ W_a0.T + b_a0)
        acts = np.concatenate([at_, ao_], axis=-1)
        srep = np.broadcast_to(states[t_, b_], (A, H))
        x = np.concatenate([srep, acts, srep * acts], axis=-1)
        x = relu(x @ W_p0.T + b_p0)
        x = relu(x @ W_p1.T + b_p1)
        x = relu(x @ W_p2.T + b_p2)
        lg = x @ W_p3.T + b_p3
        action[t_, b_] = int(np.argmax(lg[:, 0]))
    return action


# revision 29
# speedup vs baseline: 1.1619x; 1.0047x over previous
"""Trainium2 Bass kernel for nn_Net_21947282882692 (segment_reduce).

Strategy (8 NeuronCores, SPMD):
  - Data-parallel over the T*B*A segment axis == T axis (each core owns 8 of
    64 timesteps => 65536 of 524288 actions_table rows, 16384 segments).
  - The tiny MLP/LSTM params are replicated; every core runs the full
    T=64-step LSTM scan (cheap, latency-bound) interleaved with streaming
    its actions_table shard (DMA/PE-bound) through the shared t_fc tower and
    the offset-based segment sum (offset==4 => sum of 4 consecutive rows).
  - The final pointwise MLP + argmax + baseline run on each core's shard.
  - Host gathers shards along T.
"""

import sys

for _p in ("/opt/pypackages", "/opt/trn_rl_repo"):
    if _p not in sys.path:
        sys.path.insert(0, _p)

from contextlib import ExitStack

import numpy as np
import concourse.bass as bass
import concourse.mybir as mybir
from concourse import bacc
from concourse.tile import TileContext
from concourse.bass_utils import run_bass_kernel_spmd

F32 = mybir.dt.float32
F32R = mybir.dt.float32r
BF16 = mybir.dt.bfloat16
I32 = mybir.dt.int32
AF = mybir.ActivationFunctionType
ALU = mybir.AluOpType
AX = mybir.AxisListType

T, B, A = 64, 32, 64
TD, SD, AD = 128, 64, 16
K = 4
S = T * B * A            # 131072 segments
N = S * K                # 524288 ragged rows
H = 64
NCORES = 8
Tc = T // NCORES         # 8 timesteps per core
TBc = Tc * B             # 256 (t,b) rows per core
Sc = S // NCORES         # 16384 segments per core
Rc = N // NCORES         # 65536 actions_table rows per core

CHUNK = 2048             # actions rows per stream chunk (one per 2 steps)
SEGC = CHUNK // K        # 512 segments per chunk
NCHUNK = Rc // CHUNK     # 32
MCH = 512                # MLP chunk (segments)
NMCH = Sc // MCH         # 32


def _build_program(mask_steps):
    nc = bacc.Bacc("TRN2", target_bir_lowering=False, debug=False,
                   num_devices=NCORES)

    def din(name, shape, dt=F32):
        return nc.dram_tensor(name, shape, dt, kind="ExternalInput")

    def dout(name, shape, dt=F32):
        return nc.dram_tensor(name, shape, dt, kind="ExternalOutput")

    atT = din("atT", [TD, Rc])
    aoT = din("aoT", [AD, Sc])
    stT = din("stT", [TD, T * B])
    soT = din("soT", [SD, T * B])
    h0T = din("h0T", [2 * H, B])
    c0T = din("c0T", [2 * H, B])
    ndT = din("ndT", [128, T * B])
    Wt0T = din("Wt0T", [TD, 128]); bt0 = din("bt0", [128, 1])
    Wt1T = din("Wt1T", [128, 32]); bt1r4 = din("bt1r4", [128, 1])
    Ws0T = din("Ws0T", [SD, 32]); bs0 = din("bs0", [32, 1])
    Wa0T = din("Wa0T", [AD, 32]); ba0 = din("ba0", [32, 1])
    Wl = {}
    for l in (0, 1):
        for ch in ("a", "b"):
            Wl[l, ch] = din(f"Wc{l}{ch}", [128, 128])
        Wl[l, "ba"] = din(f"bls{l}a", [128, 1])
        Wl[l, "bb"] = din(f"bls{l}b", [128, 1])
    Wp0aT = din("Wp0aT", [H, 128])
    Wp0sT = din("Wp0sT", [H, 128])
    Wp0pT = din("Wp0pT", [H, 128])
    bp0 = din("bp0", [128, 1])
    Wp1T = din("Wp1T", [128, 128]); bp1 = din("bp1", [128, 1])
    Wp2T = din("Wp2T", [128, 64]); bp2 = din("bp2", [64, 1])
    wp3T = din("wp3T", [64, 1]); bp3 = din("bp3", [1, 1])
    Wb0T = din("Wb0T", [H, 64]); bb0 = din("bb0", [64, 1])
    wb1T = din("wb1T", [64, 1]); bb1 = din("bb1", [1, 1])
    i4 = din("i4", [128, 32])
    iota = din("iotaA", [64, 4 * A])

    out_logits = dout("out_logits", [Sc])
    out_action = dout("out_action", [TBc], I32)
    out_baseline = dout("out_baseline", [TBc])
    out_h = dout("out_h", [2, B, H])
    out_c = dout("out_c", [2, B, H])
    state_scr = nc.dram_tensor("state_scr", [H, T * B], F32R)

    with TileContext(nc) as tc:
        with tc.tile_pool(name="consts", bufs=1) as cpool, \
             tc.tile_pool(name="big", bufs=1) as big:

            _dmaq = [nc.sync, nc.scalar]
            _dmaqi = [0]

            def ctile(dram, shape, dt=F32):
                t = cpool.tile(shape, dt, tag=dram.name)
                src = dram[:] if dt != F32R else dram[:].bitcast(F32R)
                eng = _dmaq[_dmaqi[0] % 2]
                _dmaqi[0] += 1
                eng.dma_start(t[:], src)
                return t

            scan_ctx = ExitStack()
            scan_pool = scan_ctx.enter_context(
                tc.tile_pool(name="scan", bufs=1))
            X0 = scan_pool.tile([128, T * B + B], F32)
            X1 = scan_pool.tile([128, T * B + B], F32)
            sa64 = big.tile([H, Sc], F32R)

            # state-tower inputs first: they gate the whole scan
            st_ctx = ExitStack()
            stps = st_ctx.enter_context(
                tc.tile_pool(name="st_ps", bufs=2, space="PSUM"))
            stsb = st_ctx.enter_context(tc.tile_pool(name="st_sb", bufs=1))
            stx = stsb.tile([TD, T * B], F32R, tag="stx")
            nc.sync.dma_start(stx[:], stT[:].bitcast(F32R))
            sox = stsb.tile([SD, T * B], F32R, tag="sox")
            nc.scalar.dma_start(sox[:], soT[:].bitcast(F32R))
            wt0 = ctile(Wt0T, [TD, 128], F32R)
            bt0t = ctile(bt0, [128, 1])
            wt1 = ctile(Wt1T, [128, 32], F32R)
            bt1t = ctile(bt1r4, [128, 1])
            ws0 = ctile(Ws0T, [SD, 32], F32R)
            bs0t = ctile(bs0, [32, 1])
            wl = {}
            for l in (0, 1):
                for ch in ("a", "b"):
                    wl[l, ch] = ctile(Wl[l, ch], [128, 128])
                wl[l, "ba"] = ctile(Wl[l, "ba"], [128, 1])
                wl[l, "bb"] = ctile(Wl[l, "bb"], [128, 1])

            # ---------------- state tower ----------------
            if True:
                st1 = stsb.tile([128, T * B], F32R)
                for cidx in range(0, 4):
                    sl = bass.ts(cidx, 512)
                    p = stps.tile([128, 512], F32, tag="p")
                    nc.tensor.matmul(p[:], wt0[:], stx[:, sl], start=True,
                                     stop=True)
                    nc.scalar.activation(st1[:, sl], p[:], AF.Relu,
                                         bias=bt0t[:])
                for cidx in range(4):
                    sl = bass.ts(cidx, 512)
                    p2 = stps.tile([32, 512], F32, tag="p2")
                    nc.tensor.matmul(p2[:], wt1[:], st1[:, sl], start=True,
                                     stop=True)
                    nc.scalar.activation(X0[0:32, sl], p2[:], AF.Relu,
                                         bias=bt1t[0:32, :])
                    p3 = stps.tile([32, 512], F32, tag="p3")
                    nc.tensor.matmul(p3[:], ws0[:], sox[:, sl], start=True,
                                     stop=True)
                    nc.scalar.activation(X0[32:64, sl], p3[:], AF.Relu,
                                         bias=bs0t[:])
            st_ctx.close()

            # remaining constants (not needed until mid-scan / tail)
            wa0 = ctile(Wa0T, [AD, 32], F32R)
            ba0t = ctile(ba0, [32, 1])
            wp0a = ctile(Wp0aT, [H, 128], F32R)
            wp0s = ctile(Wp0sT, [H, 128], F32R)
            wp0p = ctile(Wp0pT, [H, 128], F32R)
            bp0t = ctile(bp0, [128, 1])
            wp1 = ctile(Wp1T, [128, 128], F32R); bp1t = ctile(bp1, [128, 1])
            wp2 = ctile(Wp2T, [128, 64], F32R); bp2t = ctile(bp2, [64, 1])
            wp3 = ctile(wp3T, [64, 1], F32R); bp3t = ctile(bp3, [1, 1])
            wb0 = ctile(Wb0T, [H, 64], F32R); bb0t = ctile(bb0, [64, 1])
            wb1 = ctile(wb1T, [64, 1], F32R); bb1t = ctile(bb1, [1, 1])
            i4t = ctile(i4, [128, 32], F32R)
            iot = ctile(iota, [64, 4 * A])
            ndt = ctile(ndT, [128, T * B]) if mask_steps else None

            # -------- interleaved LSTM scan + actions stream --------
            sctx = ExitStack()
            lstm_ps = sctx.enter_context(
                tc.tile_pool(name="lstm_ps", bufs=1, space="PSUM"))
            lstm_sb = sctx.enter_context(tc.tile_pool(name="lstm_sb", bufs=3))
            cst = sctx.enter_context(tc.tile_pool(name="cstate", bufs=2))
            xb_pool = sctx.enter_context(tc.tile_pool(name="xb", bufs=3))
            r1_pool = sctx.enter_context(tc.tile_pool(name="r1", bufs=2))
            f4_pool = sctx.enter_context(tc.tile_pool(name="f4", bufs=2))
            sps = sctx.enter_context(
                tc.tile_pool(name="sA", bufs=2, space="PSUM"))
            sps4 = sctx.enter_context(
                tc.tile_pool(name="s4", bufs=1, space="PSUM"))
            sps32 = sctx.enter_context(
                tc.tile_pool(name="s32", bufs=1, space="PSUM"))

            cprev = [None, None]
            nc.sync.dma_start(X0[64:128, 0:B], h0T[0:H, :])
            nc.sync.dma_start(X1[64:128, 0:B], h0T[0:H, :])
            nc.sync.dma_start(X1[0:64, 0:B], h0T[H:2 * H, :])
            for l in (0, 1):
                ct = cst.tile([H, B], F32, tag=f"c{l}")
                nc.sync.dma_start(ct[:], c0T[l * H:(l + 1) * H, :])
                cprev[l] = ct[:]

            def lstm_cell(l, t):
                stk = X0 if l == 0 else X1
                if t in mask_steps and l == 0:
                    for tile_, lo, hi in ((X0, 64, 128), (X1, 0, 64),
                                          (X1, 64, 128)):
                        nc.vector.tensor_tensor(
                            tile_[lo:hi, bass.ts(t, B)],
                            tile_[lo:hi, bass.ts(t, B)],
                            ndt[lo:hi, bass.ts(t, B)], op=ALU.mult)
                if t in mask_steps:
                    cm = cst.tile([H, B], F32, tag=f"cm{l}")
                    nc.vector.tensor_tensor(cm[:], cprev[l],
                                            ndt[0:H, bass.ts(t, B)],
                                            op=ALU.mult)
                    cprev[l] = cm[:]
                cp = cprev[l]
                stk_ap = stk[:, bass.ts(t, B)]
                pg = lstm_ps.tile([128, 2 * B], F32, tag=f"pg{l}")
                pa, pb = pg[:, 0:B], pg[:, B:2 * B]
                nc.tensor.matmul(pa, wl[l, "a"][:], stk_ap,
                                 start=True, stop=True)
                nc.tensor.matmul(pb, wl[l, "b"][:], stk_ap,
                                 start=True, stop=True)
                # critical-path order: f first (feeds r_), then g, i, o
                sf = lstm_sb.tile([H, B], F32, tag=f"sf{l}")
                nc.scalar.activation(sf[:], pa[H:128], AF.Sigmoid,
                                     bias=wl[l, "ba"][H:128, :])
                tg = lstm_sb.tile([H, B], F32, tag=f"tg{l}")
                nc.scalar.activation(tg[:], pb[0:H], AF.Tanh,
                                     bias=wl[l, "bb"][0:H, :])
                si = lstm_sb.tile([H, B], F32, tag=f"si{l}")
                nc.scalar.activation(si[:], pa[0:H], AF.Sigmoid,
                                     bias=wl[l, "ba"][0:H, :])
                so_ = lstm_sb.tile([H, B], F32, tag=f"so{l}")
                nc.scalar.activation(so_[:], pb[H:128], AF.Sigmoid,
                                     bias=wl[l, "bb"][H:128, :])
                r_ = lstm_sb.tile([H, B], F32, tag=f"r{l}")
                nc.gpsimd.tensor_tensor(r_[:], sf[:], cp, op=ALU.mult)
                p_ = lstm_sb.tile([H, B], F32, tag=f"p{l}")
                nc.gpsimd.tensor_tensor(p_[:], si[:], tg[:], op=ALU.mult)
                cn = cst.tile([H, B], F32, tag=f"c{l}")
                nc.gpsimd.tensor_tensor(cn[:], r_[:], p_[:], op=ALU.add)
                th = lstm_sb.tile([H, B], F32, tag=f"th{l}")
                nc.scalar.activation(th[:], cn[:], AF.Tanh)
                if l == 0:
                    hdst = X0[64:128, bass.ts(t + 1, B)]
                    nc.gpsimd.tensor_tensor(hdst, so_[:], th[:], op=ALU.mult)
                    nc.gpsimd.tensor_copy(X1[64:128, bass.ts(t, B)], hdst)
                else:
                    hdst = X1[0:64, bass.ts(t + 1, B)]
                    nc.gpsimd.tensor_tensor(hdst, so_[:], th[:], op=ALU.mult)
                cprev[l] = cn[:]

            def stream_half(k, half):
                if half == 0:
                    xb = xb_pool.tile([TD, CHUNK], F32R, tag="xb")
                    nc.sync.dma_start(xb[:],
                                      atT[:, bass.ts(k, CHUNK)].bitcast(F32R))
                    r1 = r1_pool.tile([128, CHUNK], F32R, tag="r1")
                    stream_state[k] = (xb, r1)
                xb, r1 = stream_state[k]
                for h_ in (0, 1) if half == 0 else (2, 3):
                    sl = bass.ts(h_, 512)
                    p1 = sps.tile([128, 512], F32, tag="p1")
                    nc.tensor.matmul(p1[:], wt0[:], xb[:, sl], start=True,
                                     stop=True)
                    if h_ % 2 == 0:
                        nc.scalar.activation(r1[:, sl], p1[:],
                                             AF.Relu, bias=bt0t[:])
                    else:
                        nc.vector.tensor_scalar(r1[:, sl], p1[:],
                                                bt0t[:], 0.0, op0=ALU.add,
                                                op1=ALU.max)
                if half == 0:
                    return
                r1v = r1[:].rearrange("p (s k) -> p s k", k=K)
                f4 = f4_pool.tile([128, SEGC], F32R, tag="f4")
                for b_ in range(K):
                    p4 = sps4.tile([32, SEGC], F32, tag=f"p4{b_ % 2}")
                    nc.tensor.matmul(p4[:], wt1[:], r1v[:, :, b_],
                                     start=True, stop=True)
                    dst = f4[bass.ts(b_, 32), :]
                    nc.vector.tensor_scalar(dst, p4[:], bt1t[0:32, :], 0.0,
                                            op0=ALU.add, op1=ALU.max)
                p32 = sps32.tile([32, SEGC], F32, tag="p32")
                nc.tensor.matmul(p32[:], i4t[:], f4[:], start=True, stop=True)
                nc.vector.tensor_scalar(sa64[0:32, bass.ts(k, SEGC)], p32[:],
                                        0.0, None, op0=ALU.add)

            ao_sb = sctx.enter_context(tc.tile_pool(name="ao_sb", bufs=3))

            def ao_chunk(k):
                sl = bass.ts(k, SEGC)
                aox = ao_sb.tile([AD, SEGC], F32R, tag="aox")
                nc.sync.dma_start(aox[:], aoT[:, sl].bitcast(F32R))
                p = sps32.tile([32, SEGC], F32, tag="pao")
                nc.tensor.matmul(p[:], wa0[:], aox[:], start=True, stop=True)
                if k % 2 == 0:
                    nc.scalar.activation(sa64[32:64, sl], p[:], AF.Relu,
                                         bias=ba0t[:])
                else:
                    nc.vector.tensor_scalar(sa64[32:64, sl], p[:], ba0t[:],
                                            0.0, op0=ALU.add, op1=ALU.max)

            stream_state = {}
            for t in range(T + 1):
                if t < T:
                    lstm_cell(0, t)
                if t > 0:
                    lstm_cell(1, t - 1)
                if t < T:
                    stream_half(t // 2, t % 2)
                    if t % 2 == 0:
                        ao_chunk(t // 2)

            hprev = [X0[64:128, bass.ts(T, B)], X1[0:64, bass.ts(T, B)]]
            # final h/c back to [B, H] layout and out.
            # blockwise DVE transpose keeps partition offsets aligned;
            # the DMA access pattern undoes the block structure.
            with tc.tile_pool(name="hc_out", bufs=1) as hc:
                for idx, (pair, dst) in enumerate(
                        ((hprev, out_h), (cprev, out_c))):
                    for l in (0, 1):
                        ap = pair[l]
                        tmp = hc.tile([H, B], F32, tag=f"nat{idx}{l}")
                        nc.vector.transpose(tmp[0:32, :], ap[0:32, :])
                        nc.vector.transpose(tmp[32:64, :], ap[32:64, :])
                        # tmp[32g+i, j] == h[feature 32g+j, batch i]
                        for g in range(2):
                            nc.sync.dma_start(
                                dst[l][:, bass.ts(g, 32)],
                                tmp[bass.ts(g, 32), :])

            sctx.close()

            # ---- state shard extraction (SPMD via partition id) ----
            nc.sync.dma_start(state_scr[:],
                              X1[0:64, B:B + T * B].bitcast(F32R))
            scan_ctx.close()
            with tc.tile_pool(name="shard_sb", bufs=1) as shsb:
                shard = shsb.tile([H, TBc], F32R)
                pid = nc.sync.partition_id()
                nc.sync.dma_start(shard[:],
                                  state_scr[:, bass.ds(pid * TBc, TBc)])

                # ---- final MLP over the shard ----
                with tc.tile_pool(name="mlp_sb", bufs=2) as msb, \
                     tc.tile_pool(name="m_ps", bufs=2, space="PSUM") as mps, \
                     tc.tile_pool(name="m_ps2", bufs=2, space="PSUM") as mps2, \
                     tc.tile_pool(name="m_psL", bufs=2, space="PSUM") as mpsL:
                    for m in range(NMCH):
                        sl = bass.ts(m, MCH)
                        ntb = MCH // A
                        shb = shard[:, bass.ts(m, ntb)].broadcast_to(
                            [H, ntb, A])
                        prod = msb.tile([H, MCH], F32R, tag="prod")
                        nc.gpsimd.tensor_tensor(
                            prod[:].rearrange("p (g a) -> p g a", a=A),
                            sa64[:, sl], shb, op=ALU.mult)
                        p0 = mps.tile([128, MCH], F32, tag="p0")
                        nc.tensor.matmul(p0[:], wp0a[:], sa64[:, sl],
                                         start=True, stop=False)
                        nc.tensor.matmul(p0[:], wp0p[:], prod[:], start=False,
                                         stop=False)
                        nc.tensor.matmul(p0[:], wp0s[:], shb, start=False,
                                         stop=True)
                        x1 = msb.tile([128, MCH], F32R, tag="x1")
                        if m % 2 == 0:
                            nc.scalar.activation(x1[:], p0[:],
                                                 AF.Relu, bias=bp0t[:])
                        else:
                            nc.vector.tensor_scalar(x1[:], p0[:],
                                                    bp0t[:], 0.0, op0=ALU.add,
                                                    op1=ALU.max)
                        p1_ = mps.tile([128, MCH], F32, tag="p1")
                        nc.tensor.matmul(p1_[:], wp1[:], x1[:], start=True,
                                         stop=True)
                        x2 = msb.tile([128, MCH], F32R, tag="x2")
                        if m % 2 == 1:
                            nc.scalar.activation(x2[:], p1_[:],
                                                 AF.Relu, bias=bp1t[:])
                        else:
                            nc.vector.tensor_scalar(x2[:], p1_[:],
                                                    bp1t[:], 0.0, op0=ALU.add,
                                                    op1=ALU.max)
                        p2_ = mps2.tile([64, MCH], F32, tag="p2")
                        nc.tensor.matmul(p2_[:], wp2[:], x2[:], start=True,
                                         stop=True)
                        x3 = msb.tile([64, MCH], F32R, tag="x3")
                        if m % 2 == 0:
                            nc.scalar.activation(x3[:], p2_[:],
                                                 AF.Relu, bias=bp2t[:])
                        else:
                            nc.vector.tensor_scalar(x3[:], p2_[:],
                                                    bp2t[:], 0.0, op0=ALU.add,
                                                    op1=ALU.max)
                        pL = mpsL.tile([1, MCH], F32, tag="pL")
                        nc.tensor.matmul(pL[:], wp3[:], x3[:], start=True,
                                         stop=True)
                        lgc = msb.tile([1, MCH], F32, tag="lgc")
                        if m % 2 == 0:
                            nc.scalar.activation(lgc[:], pL[:],
                                                 AF.Identity, bias=bp3t[:])
                        else:
                            nc.vector.tensor_scalar(lgc[:], pL[:],
                                                    bp3t[:], None, op0=ALU.add)
                        nc.sync.dma_start(
                            out_logits[bass.ts(m, MCH)].rearrange(
                                "(a b) -> a b", a=1), lgc[:])

                # ---- baseline ----
                with tc.tile_pool(name="bl", bufs=1) as blsb, \
                     tc.tile_pool(name="bl_ps", bufs=1, space="PSUM") as blps:
                    pb_ = blps.tile([64, TBc], F32, tag="pb")
                    nc.tensor.matmul(pb_[:], wb0[:], shard[:], start=True,
                                     stop=True)
                    bl1 = blsb.tile([64, TBc], F32R)
                    nc.scalar.activation(bl1[:], pb_[:], AF.Relu,
                                         bias=bb0t[:])
                    pb2 = blps.tile([1, TBc], F32, tag="pb2")
                    nc.tensor.matmul(pb2[:], wb1[:], bl1[:], start=True,
                                     stop=True)
                    blo = blsb.tile([1, TBc], F32)
                    nc.scalar.activation(blo[:], pb2[:], AF.Identity,
                                         bias=bb1t[:])
                    nc.sync.dma_start(
                        out_baseline[:].rearrange("(a b) -> a b", a=1),
                        blo[0:1, :])

                # ---- argmax over A per (t,b) row ----
                with tc.tile_pool(name="am", bufs=1) as am:
                    lgT = am.tile([64, TBc], F32)
                    nc.sync.dma_start(
                        lgT[:], out_logits[:].rearrange("(p f) -> p f", p=64))
                    lgv = lgT[:].rearrange("p (g a) -> p g a", a=A)
                    mx = am.tile([64, 4], F32)
                    nc.vector.tensor_reduce(mx[:], lgv, axis=AX.X, op=ALU.max)
                    eq = am.tile([64, 4 * A], F32)
                    nc.vector.tensor_tensor(
                        eq[:].rearrange("p (g a) -> p g a", a=A), lgv,
                        mx[:].broadcast_to([64, 4, A]), op=ALU.is_ge)
                    pr = am.tile([64, 4 * A], F32)
                    nc.vector.tensor_tensor(pr[:], eq[:], iot[:], op=ALU.mult)
                    idxf = am.tile([64, 4], F32)
                    nc.vector.tensor_reduce(
                        idxf[:], pr[:].rearrange("p (g a) -> p g a", a=A),
                        axis=AX.X, op=ALU.add)
                    idxi = am.tile([64, 4], I32)
                    nc.vector.tensor_copy(idxi[:], idxf[:])
                    nc.sync.dma_start(
                        out_action[:].rearrange("(p f) -> p f", p=64), idxi[:])

    nc.finalize()
    return nc


_CACHE = {}


def _get_program(mask_steps):
    key = tuple(mask_steps)
    if key not in _CACHE:
        _CACHE[key] = _build_program(key)
    return _CACHE[key]


def kernel(**inputs):
    inp = {k: np.asarray(v) for k, v in inputs.items()}
    offset = inp["offset"]
    assert offset.sum() == N, "unsupported ragged layout"
    assert not np.any(offset != K), "general-offset path not implemented"
    done = inp["done"].astype(bool)
    mask_steps = tuple(int(t) for t in range(T) if done[t].any())
    nc = _get_program(mask_steps)

    f32 = np.float32
    at = np.ascontiguousarray(inp["actions_table"], dtype=f32)
    ao = np.ascontiguousarray(inp["actions_other"], dtype=f32).reshape(S, AD)
    stT = np.ascontiguousarray(inp["state_table"].reshape(T * B, TD).T,
                               dtype=f32)
    soT = np.ascontiguousarray(inp["state_other"].reshape(T * B, SD).T,
                               dtype=f32)
    h0T = np.ascontiguousarray(
        np.concatenate([inp["h0"][0].T, inp["h0"][1].T], axis=0), dtype=f32)
    c0T = np.ascontiguousarray(
        np.concatenate([inp["c0"][0].T, inp["c0"][1].T], axis=0), dtype=f32)
    nd = (1.0 - done.astype(f32)).reshape(1, T * B)
    ndT = np.ascontiguousarray(np.broadcast_to(nd, (128, T * B)), dtype=f32)

    def tp(x):
        return np.ascontiguousarray(np.asarray(x, dtype=f32).T)

    def col(x):
        return np.ascontiguousarray(np.asarray(x, dtype=f32).reshape(-1, 1))

    com = {
        "stT": stT, "soT": soT, "h0T": h0T, "c0T": c0T, "ndT": ndT,
        "Wt0T": tp(inp["W_t0"]), "bt0": col(inp["b_t0"]),
        "Wt1T": tp(inp["W_t1"]), "bt1r4": col(np.tile(inp["b_t1"], 4)),
        "Ws0T": tp(inp["W_s0"]), "bs0": col(inp["b_s0"]),
        "Wa0T": tp(inp["W_a0"]), "ba0": col(inp["b_a0"]),
        "Wp0aT": tp(inp["W_p0"][:, 64:128]),
        "Wp0sT": tp(inp["W_p0"][:, 0:64]),
        "Wp0pT": tp(inp["W_p0"][:, 128:192]),
        "bp0": col(inp["b_p0"]),
        "Wp1T": tp(inp["W_p1"]), "bp1": col(inp["b_p1"]),
        "Wp2T": tp(inp["W_p2"]), "bp2": col(inp["b_p2"]),
        "wp3T": tp(inp["W_p3"]), "bp3": col(inp["b_p3"]),
        "Wb0T": tp(inp["W_b0"]), "bb0": col(inp["b_b0"]),
        "wb1T": tp(inp["W_b1"]), "bb1": col(inp["b_b1"]),
        "i4": np.tile(np.eye(32, dtype=f32), (4, 1)),
        "iotaA": np.tile(np.arange(A, dtype=f32), (64, 4)),
    }
    for l in (0, 1):
        wih = inp[f"Wih{l}"].astype(f32)
        whh = inp[f"Whh{l}"].astype(f32)
        if l == 0:
            wcat = np.concatenate([wih, whh], axis=1)     # [256, 128]
        else:
            wcat = np.concatenate([whh, wih], axis=1)     # stack is [h1; h0]
        bls = (inp[f"bih{l}"] + inp[f"bhh{l}"]).astype(f32)
        com[f"Wc{l}a"] = tp(wcat[0:128, :])
        com[f"Wc{l}b"] = tp(wcat[128:256, :])
        com[f"bls{l}a"] = col(bls[0:128])
        com[f"bls{l}b"] = col(bls[128:256])

    in_maps = []
    for i in range(NCORES):
        m = dict(com)
        m["atT"] = np.ascontiguousarray(at[i * Rc:(i + 1) * Rc].T)
        m["aoT"] = np.ascontiguousarray(ao[i * Sc:(i + 1) * Sc].T)
        in_maps.append(m)

    res = run_bass_kernel_spmd(nc, in_maps, list(range(NCORES)), trace=False)
    r = res.results

    logits = np.concatenate([r[i]["out_logits"] for i in range(NCORES)])
    logits = logits.reshape(T, B, A)
    baseline = np.concatenate(
        [r[i]["out_baseline"] for i in range(NCORES)]).reshape(T, B)
    action = np.concatenate(
        [r[i]["out_action"] for i in range(NCORES)]).reshape(T, B)
    action = _refine_ties(inp, logits, action.astype(np.int32))
    hT = r[0]["out_h"]
    cT = r[0]["out_c"]
    return logits, baseline, action, hT, cT


def _refine_ties(inp, logits, action):
    """f32r matmuls carry ~1e-4 relative noise; rows whose top-2 logit gap is
    below that can argmax differently than fp32. Recompute just those rows
    in float64 on the host (a handful of rows, ~10 MFLOP each)."""
    srt = np.sort(logits, axis=-1)
    gap = srt[..., -1] - srt[..., -2]
    thr = 1e-2 * np.abs(logits).max()
    rows = np.argwhere(gap < thr)
    if rows.size == 0:
        return action
    f64 = np.float64
    W_t0 = inp["W_t0"].astype(f64); b_t0 = inp["b_t0"].astype(f64)
    W_t1 = inp["W_t1"].astype(f64); b_t1 = inp["b_t1"].astype(f64)

    # full LSTM state in f64 (cheap: ~4 MFLOP)
    def relu(x):
        return np.maximum(x, 0.0)

    st = relu(relu(inp["state_table"].reshape(T * B, TD).astype(f64)
                   @ W_t0.T + b_t0) @ W_t1.T + b_t1)
    so = relu(inp["state_other"].reshape(T * B, SD).astype(f64)
              @ inp["W_s0"].astype(f64).T + inp["b_s0"].astype(f64))
    core_in = np.concatenate([st, so], axis=-1).reshape(T, B, H)
    notdone = 1.0 - inp["done"].astype(f64)
    h = inp["h0"].astype(f64).copy(); c = inp["c0"].astype(f64).copy()
    Wih = [inp["Wih0"].astype(f64), inp["Wih1"].astype(f64)]
    Whh = [inp["Whh0"].astype(f64), inp["Whh1"].astype(f64)]
    bl_ = [(inp["bih0"] + inp["bhh0"]).astype(f64),
           (inp["bih1"] + inp["bhh1"]).astype(f64)]

    def sig(x):
        return 1.0 / (1.0 + np.exp(-x))

    states = np.zeros((T, B, H), f64)
    for t in range(T):
        h = h * notdone[t][None, :, None]
        c = c * notdone[t][None, :, None]
        x = core_in[t]
        for l in (0, 1):
            g = x @ Wih[l].T + h[l] @ Whh[l].T + bl_[l]
            i_, f_, g_, o_ = np.split(g, 4, axis=-1)
            c[l] = sig(f_) * c[l] + sig(i_) * np.tanh(g_)
            x = sig(o_) * np.tanh(c[l])
            h[l] = x
        states[t] = h[1]

    at_tab = inp["actions_table"].astype(f64)
    ao_all = inp["actions_other"].reshape(S, AD).astype(f64)
    W_a0 = inp["W_a0"].astype(f64); b_a0 = inp["b_a0"].astype(f64)
    W_p0 = inp["W_p0"].astype(f64); b_p0 = inp["b_p0"].astype(f64)
    W_p1 = inp["W_p1"].astype(f64); b_p1 = inp["b_p1"].astype(f64)
    W_p2 = inp["W_p2"].astype(f64); b_p2 = inp["b_p2"].astype(f64)
    W_p3 = inp["W_p3"].astype(f64); b_p3 = inp["b_p3"].astype(f64)
    offs = inp["offset"].reshape(-1)
    cum = np.zeros(S + 1, np.int64)
    np.cumsum(offs, out=cum[1:])
    for t_, b_ in rows:
        tb = t_ * B + b_
        segs = tb * A + np.arange(A)
        at_rows = []
        for s_ in segs:
            rws = at_tab[cum[s_]:cum[s_ + 1]]
            ft = relu(relu(rws @ W_t0.T + b_t0) @ W_t1.T + b_t1)
            at_rows.append(ft.sum(axis=0))
        at_ = np.stack(at_rows)
        ao_ = relu(ao_all[segs] @ W_a0.T + b_a0)
        acts = np.concatenate([at_, ao_], axis=-1)
        srep = np.broadcast_to(states[t_, b_], (A, H))
        x = np.concatenate([srep, acts, srep * acts], axis=-1)
        x = relu(x @ W_p0.T + b_p0)
        x = relu(x @ W_p1.T + b_p1)
        x = relu(x @ W_p2.T + b_p2)
        lg = x @ W_p3.T + b_p3
        action[t_, b_] = int(np.argmax(lg[:, 0]))
    return action


# revision 30
# speedup vs baseline: 1.1741x; 1.0105x over previous
"""Trainium2 Bass kernel for nn_Net_21947282882692 (segment_reduce).

Strategy (8 NeuronCores, SPMD):
  - Data-parallel over the T*B*A segment axis == T axis (each core owns 8 of
    64 timesteps => 65536 of 524288 actions_table rows, 16384 segments).
  - The tiny MLP/LSTM params are replicated; every core runs the full
    T=64-step LSTM scan (cheap, latency-bound) interleaved with streaming
    its actions_table shard (DMA/PE-bound) through the shared t_fc tower and
    the offset-based segment sum (offset==4 => sum of 4 consecutive rows).
  - The final pointwise MLP + argmax + baseline run on each core's shard.
  - Host gathers shards along T.
"""

import sys

for _p in ("/opt/pypackages", "/opt/trn_rl_repo"):
    if _p not in sys.path:
        sys.path.insert(0, _p)

from contextlib import ExitStack

import numpy as np
import concourse.bass as bass
import concourse.mybir as mybir
from concourse import bacc
from concourse.tile import TileContext
from concourse.bass_utils import run_bass_kernel_spmd

F32 = mybir.dt.float32
F32R = mybir.dt.float32r
BF16 = mybir.dt.bfloat16
I32 = mybir.dt.int32
AF = mybir.ActivationFunctionType
ALU = mybir.AluOpType
AX = mybir.AxisListType

T, B, A = 64, 32, 64
TD, SD, AD = 128, 64, 16
K = 4
S = T * B * A            # 131072 segments
N = S * K                # 524288 ragged rows
H = 64
NCORES = 8
Tc = T // NCORES         # 8 timesteps per core
TBc = Tc * B             # 256 (t,b) rows per core
Sc = S // NCORES         # 16384 segments per core
Rc = N // NCORES         # 65536 actions_table rows per core

CHUNK = 2048             # actions rows per stream chunk (one per 2 steps)
SEGC = CHUNK // K        # 512 segments per chunk
NCHUNK = Rc // CHUNK     # 32
MCH = 512                # MLP chunk (segments)
NMCH = Sc // MCH         # 32


def _build_program(mask_steps):
    nc = bacc.Bacc("TRN2", target_bir_lowering=False, debug=False,
                   num_devices=NCORES)

    def din(name, shape, dt=F32):
        return nc.dram_tensor(name, shape, dt, kind="ExternalInput")

    def dout(name, shape, dt=F32):
        return nc.dram_tensor(name, shape, dt, kind="ExternalOutput")

    atT = din("atT", [TD, Rc])
    aoT = din("aoT", [AD, Sc])
    stT = din("stT", [TD, T * B])
    soT = din("soT", [SD, T * B])
    h0T = din("h0T", [2 * H, B])
    c0T = din("c0T", [2 * H, B])
    ndT = din("ndT", [128, T * B])
    Wt0T = din("Wt0T", [TD, 128]); bt0 = din("bt0", [128, 1])
    Wt1T = din("Wt1T", [128, 32]); bt1r4 = din("bt1r4", [128, 1])
    Ws0T = din("Ws0T", [SD, 32]); bs0 = din("bs0", [32, 1])
    Wa0T = din("Wa0T", [AD, 32]); ba0 = din("ba0", [32, 1])
    Wl = {}
    for l in (0, 1):
        for ch in ("a", "b"):
            Wl[l, ch] = din(f"Wc{l}{ch}", [128, 128])
        Wl[l, "ba"] = din(f"bls{l}a", [128, 1])
        Wl[l, "bb"] = din(f"bls{l}b", [128, 1])
    Wp0aT = din("Wp0aT", [H, 128])
    Wp0sT = din("Wp0sT", [H, 128])
    Wp0pT = din("Wp0pT", [H, 128])
    bp0 = din("bp0", [128, 1])
    Wp1T = din("Wp1T", [128, 128]); bp1 = din("bp1", [128, 1])
    Wp2T = din("Wp2T", [128, 64]); bp2 = din("bp2", [64, 1])
    wp3T = din("wp3T", [64, 1]); bp3 = din("bp3", [1, 1])
    Wb0T = din("Wb0T", [H, 64]); bb0 = din("bb0", [64, 1])
    wb1T = din("wb1T", [64, 1]); bb1 = din("bb1", [1, 1])
    i4 = din("i4", [128, 32])
    iota = din("iotaA", [64, 4 * A])

    out_logits = dout("out_logits", [Sc])
    out_action = dout("out_action", [TBc], I32)
    out_baseline = dout("out_baseline", [TBc])
    out_h = dout("out_h", [2, B, H])
    out_c = dout("out_c", [2, B, H])
    state_scr = nc.dram_tensor("state_scr", [H, T * B], F32R)

    with TileContext(nc) as tc:
        with tc.tile_pool(name="consts", bufs=1) as cpool, \
             tc.tile_pool(name="big", bufs=1) as big:

            _dmaq = [nc.sync, nc.scalar]
            _dmaqi = [0]

            def ctile(dram, shape, dt=F32):
                t = cpool.tile(shape, dt, tag=dram.name)
                src = dram[:] if dt != F32R else dram[:].bitcast(F32R)
                eng = _dmaq[_dmaqi[0] % 2]
                _dmaqi[0] += 1
                eng.dma_start(t[:], src)
                return t

            scan_ctx = ExitStack()
            scan_pool = scan_ctx.enter_context(
                tc.tile_pool(name="scan", bufs=1))
            X0 = scan_pool.tile([128, T * B + B], F32)
            X1 = scan_pool.tile([128, T * B + B], F32)
            sa64 = big.tile([H, Sc], F32R)

            # state-tower inputs first: they gate the whole scan
            st_ctx = ExitStack()
            stps = st_ctx.enter_context(
                tc.tile_pool(name="st_ps", bufs=2, space="PSUM"))
            stsb = st_ctx.enter_context(tc.tile_pool(name="st_sb", bufs=1))
            stx = stsb.tile([TD, T * B], F32R, tag="stx")
            nc.sync.dma_start(stx[:], stT[:].bitcast(F32R))
            sox = stsb.tile([SD, T * B], F32R, tag="sox")
            nc.scalar.dma_start(sox[:], soT[:].bitcast(F32R))
            wt0 = ctile(Wt0T, [TD, 128], F32R)
            bt0t = ctile(bt0, [128, 1])
            wt1 = ctile(Wt1T, [128, 32], F32R)
            bt1t = ctile(bt1r4, [128, 1])
            ws0 = ctile(Ws0T, [SD, 32], F32R)
            bs0t = ctile(bs0, [32, 1])
            wl = {}
            for l in (0, 1):
                for ch in ("a", "b"):
                    wl[l, ch] = ctile(Wl[l, ch], [128, 128])
                wl[l, "ba"] = ctile(Wl[l, "ba"], [128, 1])
                wl[l, "bb"] = ctile(Wl[l, "bb"], [128, 1])

            # ---------------- state tower ----------------
            if True:
                st1 = stsb.tile([128, T * B], F32R)
                for cidx in range(0, 4):
                    sl = bass.ts(cidx, 512)
                    p = stps.tile([128, 512], F32, tag="p")
                    nc.tensor.matmul(p[:], wt0[:], stx[:, sl], start=True,
                                     stop=True)
                    nc.scalar.activation(st1[:, sl], p[:], AF.Relu,
                                         bias=bt0t[:])
                for cidx in range(4):
                    sl = bass.ts(cidx, 512)
                    p2 = stps.tile([32, 512], F32, tag="p2")
                    nc.tensor.matmul(p2[:], wt1[:], st1[:, sl], start=True,
                                     stop=True)
                    nc.scalar.activation(X0[0:32, sl], p2[:], AF.Relu,
                                         bias=bt1t[0:32, :])
                    p3 = stps.tile([32, 512], F32, tag="p3")
                    nc.tensor.matmul(p3[:], ws0[:], sox[:, sl], start=True,
                                     stop=True)
                    nc.scalar.activation(X0[32:64, sl], p3[:], AF.Relu,
                                         bias=bs0t[:])
            st_ctx.close()

            # remaining constants (not needed until mid-scan / tail)
            wa0 = ctile(Wa0T, [AD, 32], F32R)
            ba0t = ctile(ba0, [32, 1])
            wp0a = ctile(Wp0aT, [H, 128], F32R)
            wp0s = ctile(Wp0sT, [H, 128], F32R)
            wp0p = ctile(Wp0pT, [H, 128], F32R)
            bp0t = ctile(bp0, [128, 1])
            wp1 = ctile(Wp1T, [128, 128], F32R); bp1t = ctile(bp1, [128, 1])
            wp2 = ctile(Wp2T, [128, 64], F32R); bp2t = ctile(bp2, [64, 1])
            wp3 = ctile(wp3T, [64, 1], F32R); bp3t = ctile(bp3, [1, 1])
            wb0 = ctile(Wb0T, [H, 64], F32R); bb0t = ctile(bb0, [64, 1])
            wb1 = ctile(wb1T, [64, 1], F32R); bb1t = ctile(bb1, [1, 1])
            i4t = ctile(i4, [128, 32], F32R)
            iot = ctile(iota, [64, 4 * A])
            ndt = ctile(ndT, [128, T * B]) if mask_steps else None

            # -------- interleaved LSTM scan + actions stream --------
            sctx = ExitStack()
            lstm_ps = sctx.enter_context(
                tc.tile_pool(name="lstm_ps", bufs=1, space="PSUM"))
            lstm_sb = sctx.enter_context(tc.tile_pool(name="lstm_sb", bufs=3))
            cst = sctx.enter_context(tc.tile_pool(name="cstate", bufs=2))
            xb_pool = sctx.enter_context(tc.tile_pool(name="xb", bufs=3))
            r1_pool = sctx.enter_context(tc.tile_pool(name="r1", bufs=2))
            f4_pool = sctx.enter_context(tc.tile_pool(name="f4", bufs=2))
            sps = sctx.enter_context(
                tc.tile_pool(name="sA", bufs=2, space="PSUM"))
            sps4 = sctx.enter_context(
                tc.tile_pool(name="s4", bufs=1, space="PSUM"))
            sps32 = sctx.enter_context(
                tc.tile_pool(name="s32", bufs=1, space="PSUM"))

            cprev = [None, None]
            nc.sync.dma_start(X0[64:128, 0:B], h0T[0:H, :])
            nc.sync.dma_start(X1[64:128, 0:B], h0T[0:H, :])
            nc.sync.dma_start(X1[0:64, 0:B], h0T[H:2 * H, :])
            for l in (0, 1):
                ct = cst.tile([H, B], F32, tag=f"c{l}")
                nc.sync.dma_start(ct[:], c0T[l * H:(l + 1) * H, :])
                cprev[l] = ct[:]

            def lstm_cell(l, t):
                stk = X0 if l == 0 else X1
                if t in mask_steps and l == 0:
                    for tile_, lo, hi in ((X0, 64, 128), (X1, 0, 64),
                                          (X1, 64, 128)):
                        nc.vector.tensor_tensor(
                            tile_[lo:hi, bass.ts(t, B)],
                            tile_[lo:hi, bass.ts(t, B)],
                            ndt[lo:hi, bass.ts(t, B)], op=ALU.mult)
                if t in mask_steps:
                    cm = cst.tile([H, B], F32, tag=f"cm{l}")
                    nc.vector.tensor_tensor(cm[:], cprev[l],
                                            ndt[0:H, bass.ts(t, B)],
                                            op=ALU.mult)
                    cprev[l] = cm[:]
                cp = cprev[l]
                stk_ap = stk[:, bass.ts(t, B)]
                pg = lstm_ps.tile([128, 2 * B], F32, tag=f"pg{l}")
                pa, pb = pg[:, 0:B], pg[:, B:2 * B]
                nc.tensor.matmul(pa, wl[l, "a"][:], stk_ap,
                                 start=True, stop=True)
                nc.tensor.matmul(pb, wl[l, "b"][:], stk_ap,
                                 start=True, stop=True)
                # critical-path order: f first (feeds r_), then g, i, o
                sf = lstm_sb.tile([H, B], F32, tag=f"sf{l}")
                nc.scalar.activation(sf[:], pa[H:128], AF.Sigmoid,
                                     bias=wl[l, "ba"][H:128, :])
                tg = lstm_sb.tile([H, B], F32, tag=f"tg{l}")
                nc.scalar.activation(tg[:], pb[0:H], AF.Tanh,
                                     bias=wl[l, "bb"][0:H, :])
                si = lstm_sb.tile([H, B], F32, tag=f"si{l}")
                nc.scalar.activation(si[:], pa[0:H], AF.Sigmoid,
                                     bias=wl[l, "ba"][0:H, :])
                so_ = lstm_sb.tile([H, B], F32, tag=f"so{l}")
                nc.scalar.activation(so_[:], pb[H:128], AF.Sigmoid,
                                     bias=wl[l, "bb"][H:128, :])
                r_ = lstm_sb.tile([H, B], F32, tag=f"r{l}")
                nc.gpsimd.tensor_tensor(r_[:], sf[:], cp, op=ALU.mult)
                p_ = lstm_sb.tile([H, B], F32, tag=f"p{l}")
                nc.gpsimd.tensor_tensor(p_[:], si[:], tg[:], op=ALU.mult)
                cn = cst.tile([H, B], F32, tag=f"c{l}")
                nc.gpsimd.tensor_tensor(cn[:], r_[:], p_[:], op=ALU.add)
                th = lstm_sb.tile([H, B], F32, tag=f"th{l}")
                nc.scalar.activation(th[:], cn[:], AF.Tanh)
                if l == 0:
                    hdst = X0[64:128, bass.ts(t + 1, B)]
                    nc.gpsimd.tensor_tensor(hdst, so_[:], th[:], op=ALU.mult)
                    nc.gpsimd.tensor_copy(X1[64:128, bass.ts(t, B)], hdst)
                else:
                    hdst = X1[0:64, bass.ts(t + 1, B)]
                    nc.gpsimd.tensor_tensor(hdst, so_[:], th[:], op=ALU.mult)
                cprev[l] = cn[:]

            def stream_half(k, half):
                if half == 0:
                    xb = xb_pool.tile([TD, CHUNK], F32R, tag="xb")
                    nc.sync.dma_start(xb[:],
                                      atT[:, bass.ts(k, CHUNK)].bitcast(F32R))
                    r1 = r1_pool.tile([128, CHUNK], F32R, tag="r1")
                    stream_state[k] = (xb, r1)
                xb, r1 = stream_state[k]
                for h_ in (0, 1) if half == 0 else (2, 3):
                    sl = bass.ts(h_, 512)
                    p1 = sps.tile([128, 512], F32, tag="p1")
                    nc.tensor.matmul(p1[:], wt0[:], xb[:, sl], start=True,
                                     stop=True)
                    if h_ % 2 == 0:
                        nc.scalar.activation(r1[:, sl], p1[:],
                                             AF.Relu, bias=bt0t[:])
                    else:
                        nc.vector.tensor_scalar(r1[:, sl], p1[:],
                                                bt0t[:], 0.0, op0=ALU.add,
                                                op1=ALU.max)
                if half == 0:
                    return
                r1v = r1[:].rearrange("p (s k) -> p s k", k=K)
                f4 = f4_pool.tile([128, SEGC], F32R, tag="f4")
                for b_ in range(K):
                    p4 = sps4.tile([32, SEGC], F32, tag=f"p4{b_ % 2}")
                    nc.tensor.matmul(p4[:], wt1[:], r1v[:, :, b_],
                                     start=True, stop=True)
                    dst = f4[bass.ts(b_, 32), :]
                    nc.vector.tensor_scalar(dst, p4[:], bt1t[0:32, :], 0.0,
                                            op0=ALU.add, op1=ALU.max)
                p32 = sps32.tile([32, SEGC], F32, tag="p32")
                nc.tensor.matmul(p32[:], i4t[:], f4[:], start=True, stop=True)
                nc.vector.tensor_scalar(sa64[0:32, bass.ts(k, SEGC)], p32[:],
                                        0.0, None, op0=ALU.add)

            ao_sb = sctx.enter_context(tc.tile_pool(name="ao_sb", bufs=3))

            def ao_chunk(k):
                sl = bass.ts(k, SEGC)
                aox = ao_sb.tile([AD, SEGC], F32R, tag="aox")
                nc.sync.dma_start(aox[:], aoT[:, sl].bitcast(F32R))
                p = sps32.tile([32, SEGC], F32, tag="pao")
                nc.tensor.matmul(p[:], wa0[:], aox[:], start=True, stop=True)
                if k % 2 == 0:
                    nc.scalar.activation(sa64[32:64, sl], p[:], AF.Relu,
                                         bias=ba0t[:])
                else:
                    nc.vector.tensor_scalar(sa64[32:64, sl], p[:], ba0t[:],
                                            0.0, op0=ALU.add, op1=ALU.max)

            stream_state = {}
            for t in range(T + 1):
                if t < T:
                    lstm_cell(0, t)
                if t > 0:
                    lstm_cell(1, t - 1)
                if t < T:
                    stream_half(t // 2, t % 2)
                    if t % 2 == 0:
                        ao_chunk(t // 2)

            hprev = [X0[64:128, bass.ts(T, B)], X1[0:64, bass.ts(T, B)]]
            # final h/c back to [B, H] layout and out.
            # blockwise DVE transpose keeps partition offsets aligned;
            # the DMA access pattern undoes the block structure.
            with tc.tile_pool(name="hc_out", bufs=1) as hc:
                for idx, (pair, dst) in enumerate(
                        ((hprev, out_h), (cprev, out_c))):
                    for l in (0, 1):
                        ap = pair[l]
                        tmp = hc.tile([H, B], F32, tag=f"nat{idx}{l}")
                        nc.vector.transpose(tmp[0:32, :], ap[0:32, :])
                        nc.vector.transpose(tmp[32:64, :], ap[32:64, :])
                        # tmp[32g+i, j] == h[feature 32g+j, batch i]
                        for g in range(2):
                            nc.sync.dma_start(
                                dst[l][:, bass.ts(g, 32)],
                                tmp[bass.ts(g, 32), :])

            sctx.close()

            # ---- state shard extraction (SPMD via partition id) ----
            nc.sync.dma_start(state_scr[:],
                              X1[0:64, B:B + T * B].bitcast(F32R))
            scan_ctx.close()
            with tc.tile_pool(name="shard_sb", bufs=1) as shsb:
                shard = shsb.tile([H, TBc], F32R)
                pid = nc.sync.partition_id()
                nc.sync.dma_start(shard[:],
                                  state_scr[:, bass.ds(pid * TBc, TBc)])

                # ---- final MLP over the shard ----
                with tc.tile_pool(name="mlp_sb", bufs=2) as msb, \
                     tc.tile_pool(name="m_ps", bufs=2, space="PSUM") as mps, \
                     tc.tile_pool(name="m_ps2", bufs=2, space="PSUM") as mps2, \
                     tc.tile_pool(name="m_psL", bufs=2, space="PSUM") as mpsL:
                    # baseline first: it only needs the shard, so it overlaps
                    # the chunk pipeline instead of trailing it
                    pb_ = mps2.tile([64, TBc], F32, tag="p2")
                    nc.tensor.matmul(pb_[:], wb0[:], shard[:], start=True,
                                     stop=True)
                    bl1 = msb.tile([64, TBc], F32R, tag="bl1")
                    nc.scalar.activation(bl1[:], pb_[:], AF.Relu,
                                         bias=bb0t[:])
                    pb2 = mpsL.tile([1, TBc], F32, tag="pL")
                    nc.tensor.matmul(pb2[:], wb1[:], bl1[:], start=True,
                                     stop=True)
                    blo = msb.tile([1, TBc], F32, tag="blo")
                    nc.scalar.activation(blo[:], pb2[:], AF.Identity,
                                         bias=bb1t[:])
                    nc.sync.dma_start(
                        out_baseline[:].rearrange("(a b) -> a b", a=1),
                        blo[0:1, :])
                    for m in range(NMCH):
                        sl = bass.ts(m, MCH)
                        ntb = MCH // A
                        shb = shard[:, bass.ts(m, ntb)].broadcast_to(
                            [H, ntb, A])
                        prod = msb.tile([H, MCH], F32R, tag="prod")
                        nc.gpsimd.tensor_tensor(
                            prod[:].rearrange("p (g a) -> p g a", a=A),
                            sa64[:, sl], shb, op=ALU.mult)
                        p0 = mps.tile([128, MCH], F32, tag="p0")
                        nc.tensor.matmul(p0[:], wp0a[:], sa64[:, sl],
                                         start=True, stop=False)
                        nc.tensor.matmul(p0[:], wp0p[:], prod[:], start=False,
                                         stop=False)
                        nc.tensor.matmul(p0[:], wp0s[:], shb, start=False,
                                         stop=True)
                        x1 = msb.tile([128, MCH], F32R, tag="x1")
                        if m % 2 == 0:
                            nc.scalar.activation(x1[:], p0[:],
                                                 AF.Relu, bias=bp0t[:])
                        else:
                            nc.vector.tensor_scalar(x1[:], p0[:],
                                                    bp0t[:], 0.0, op0=ALU.add,
                                                    op1=ALU.max)
                        p1_ = mps.tile([128, MCH], F32, tag="p1")
                        nc.tensor.matmul(p1_[:], wp1[:], x1[:], start=True,
                                         stop=True)
                        x2 = msb.tile([128, MCH], F32R, tag="x2")
                        if m % 2 == 1:
                            nc.scalar.activation(x2[:], p1_[:],
                                                 AF.Relu, bias=bp1t[:])
                        else:
                            nc.vector.tensor_scalar(x2[:], p1_[:],
                                                    bp1t[:], 0.0, op0=ALU.add,
                                                    op1=ALU.max)
                        p2_ = mps2.tile([64, MCH], F32, tag="p2")
                        nc.tensor.matmul(p2_[:], wp2[:], x2[:], start=True,
                                         stop=True)
                        x3 = msb.tile([64, MCH], F32R, tag="x3")
                        if m % 2 == 0:
                            nc.scalar.activation(x3[:], p2_[:],
                                                 AF.Relu, bias=bp2t[:])
                        else:
                            nc.vector.tensor_scalar(x3[:], p2_[:],
                                                    bp2t[:], 0.0, op0=ALU.add,
                                                    op1=ALU.max)
                        pL = mpsL.tile([1, MCH], F32, tag="pL")
                        nc.tensor.matmul(pL[:], wp3[:], x3[:], start=True,
                                         stop=True)
                        lgc = msb.tile([1, MCH], F32, tag="lgc")
                        if m % 2 == 0:
                            nc.scalar.activation(lgc[:], pL[:],
                                                 AF.Identity, bias=bp3t[:])
                        else:
                            nc.vector.tensor_scalar(lgc[:], pL[:],
                                                    bp3t[:], None, op0=ALU.add)
                        nc.sync.dma_start(
                            out_logits[bass.ts(m, MCH)].rearrange(
                                "(a b) -> a b", a=1), lgc[:])

                # ---- argmax over A per (t,b) row ----
                with tc.tile_pool(name="am", bufs=1) as am:
                    lgT = am.tile([64, TBc], F32)
                    nc.sync.dma_start(
                        lgT[:], out_logits[:].rearrange("(p f) -> p f", p=64))
                    lgv = lgT[:].rearrange("p (g a) -> p g a", a=A)
                    mx = am.tile([64, 4], F32)
                    nc.vector.tensor_reduce(mx[:], lgv, axis=AX.X, op=ALU.max)
                    eq = am.tile([64, 4 * A], F32)
                    nc.vector.tensor_tensor(
                        eq[:].rearrange("p (g a) -> p g a", a=A), lgv,
                        mx[:].broadcast_to([64, 4, A]), op=ALU.is_ge)
                    pr = am.tile([64, 4 * A], F32)
                    nc.vector.tensor_tensor(pr[:], eq[:], iot[:], op=ALU.mult)
                    idxf = am.tile([64, 4], F32)
                    nc.vector.tensor_reduce(
                        idxf[:], pr[:].rearrange("p (g a) -> p g a", a=A),
                        axis=AX.X, op=ALU.add)
                    idxi = am.tile([64, 4], I32)
                    nc.vector.tensor_copy(idxi[:], idxf[:])
                    nc.sync.dma_start(
                        out_action[:].rearrange("(p f) -> p f", p=64), idxi[:])

    nc.finalize()
    return nc


_CACHE = {}


def _get_program(mask_steps):
    key = tuple(mask_steps)
    if key not in _CACHE:
        _CACHE[key] = _build_program(key)
    return _CACHE[key]


def kernel(**inputs):
    inp = {k: np.asarray(v) for k, v in inputs.items()}
    offset = inp["offset"]
    assert offset.sum() == N, "unsupported ragged layout"
    assert not np.any(offset != K), "general-offset path not implemented"
    done = inp["done"].astype(bool)
    mask_steps = tuple(int(t) for t in range(T) if done[t].any())
    nc = _get_program(mask_steps)

    f32 = np.float32
    at = np.ascontiguousarray(inp["actions_table"], dtype=f32)
    ao = np.ascontiguousarray(inp["actions_other"], dtype=f32).reshape(S, AD)
    stT = np.ascontiguousarray(inp["state_table"].reshape(T * B, TD).T,
                               dtype=f32)
    soT = np.ascontiguousarray(inp["state_other"].reshape(T * B, SD).T,
                               dtype=f32)
    h0T = np.ascontiguousarray(
        np.concatenate([inp["h0"][0].T, inp["h0"][1].T], axis=0), dtype=f32)
    c0T = np.ascontiguousarray(
        np.concatenate([inp["c0"][0].T, inp["c0"][1].T], axis=0), dtype=f32)
    nd = (1.0 - done.astype(f32)).reshape(1, T * B)
    ndT = np.ascontiguousarray(np.broadcast_to(nd, (128, T * B)), dtype=f32)

    def tp(x):
        return np.ascontiguousarray(np.asarray(x, dtype=f32).T)

    def col(x):
        return np.ascontiguousarray(np.asarray(x, dtype=f32).reshape(-1, 1))

    com = {
        "stT": stT, "soT": soT, "h0T": h0T, "c0T": c0T, "ndT": ndT,
        "Wt0T": tp(inp["W_t0"]), "bt0": col(inp["b_t0"]),
        "Wt1T": tp(inp["W_t1"]), "bt1r4": col(np.tile(inp["b_t1"], 4)),
        "Ws0T": tp(inp["W_s0"]), "bs0": col(inp["b_s0"]),
        "Wa0T": tp(inp["W_a0"]), "ba0": col(inp["b_a0"]),
        "Wp0aT": tp(inp["W_p0"][:, 64:128]),
        "Wp0sT": tp(inp["W_p0"][:, 0:64]),
        "Wp0pT": tp(inp["W_p0"][:, 128:192]),
        "bp0": col(inp["b_p0"]),
        "Wp1T": tp(inp["W_p1"]), "bp1": col(inp["b_p1"]),
        "Wp2T": tp(inp["W_p2"]), "bp2": col(inp["b_p2"]),
        "wp3T": tp(inp["W_p3"]), "bp3": col(inp["b_p3"]),
        "Wb0T": tp(inp["W_b0"]), "bb0": col(inp["b_b0"]),
        "wb1T": tp(inp["W_b1"]), "bb1": col(inp["b_b1"]),
        "i4": np.tile(np.eye(32, dtype=f32), (4, 1)),
        "iotaA": np.tile(np.arange(A, dtype=f32), (64, 4)),
    }
    for l in (0, 1):
        wih = inp[f"Wih{l}"].astype(f32)
        whh = inp[f"Whh{l}"].astype(f32)
        if l == 0:
            wcat = np.concatenate([wih, whh], axis=1)     # [256, 128]
        else:
            wcat = np.concatenate([whh, wih], axis=1)     # stack is [h1; h0]
        bls = (inp[f"bih{l}"] + inp[f"bhh{l}"]).astype(f32)
        com[f"Wc{l}a"] = tp(wcat[0:128, :])
        com[f"Wc{l}b"] = tp(wcat[128:256, :])
        com[f"bls{l}a"] = col(bls[0:128])
        com[f"bls{l}b"] = col(bls[128:256])

    in_maps = []
    for i in range(NCORES):
        m = dict(com)
        m["atT"] = np.ascontiguousarray(at[i * Rc:(i + 1) * Rc].T)
        m["aoT"] = np.ascontiguousarray(ao[i * Sc:(i + 1) * Sc].T)
        in_maps.append(m)

    res = run_bass_kernel_spmd(nc, in_maps, list(range(NCORES)), trace=False)
    r = res.results

    logits = np.concatenate([r[i]["out_logits"] for i in range(NCORES)])
    logits = logits.reshape(T, B, A)
    baseline = np.concatenate(
        [r[i]["out_baseline"] for i in range(NCORES)]).reshape(T, B)
    action = np.concatenate(
        [r[i]["out_action"] for i in range(NCORES)]).reshape(T, B)
    action = _refine_ties(inp, logits, action.astype(np.int32))
    hT = r[0]["out_h"]
    cT = r[0]["out_c"]
    return logits, baseline, action, hT, cT


def _refine_ties(inp, logits, action):
    """f32r matmuls carry ~1e-4 relative noise; rows whose top-2 logit gap is
    below that can argmax differently than fp32. Recompute just those rows
    in float64 on the host (a handful of rows, ~10 MFLOP each)."""
    srt = np.sort(logits, axis=-1)
    gap = srt[..., -1] - srt[..., -2]
    thr = 1e-2 * np.abs(logits).max()
    rows = np.argwhere(gap < thr)
    if rows.size == 0:
        return action
    f64 = np.float64
    W_t0 = inp["W_t0"].astype(f64); b_t0 = inp["b_t0"].astype(f64)
    W_t1 = inp["W_t1"].astype(f64); b_t1 = inp["b_t1"].astype(f64)

    # full LSTM state in f64 (cheap: ~4 MFLOP)
    def relu(x):
        return np.maximum(x, 0.0)

    st = relu(relu(inp["state_table"].reshape(T * B, TD).astype(f64)
                   @ W_t0.T + b_t0) @ W_t1.T + b_t1)
    so = relu(inp["state_other"].reshape(T * B, SD).astype(f64)
              @ inp["W_s0"].astype(f64).T + inp["b_s0"].astype(f64))
    core_in = np.concatenate([st, so], axis=-1).reshape(T, B, H)
    notdone = 1.0 - inp["done"].astype(f64)
    h = inp["h0"].astype(f64).copy(); c = inp["c0"].astype(f64).copy()
    Wih = [inp["Wih0"].astype(f64), inp["Wih1"].astype(f64)]
    Whh = [inp["Whh0"].astype(f64), inp["Whh1"].astype(f64)]
    bl_ = [(inp["bih0"] + inp["bhh0"]).astype(f64),
           (inp["bih1"] + inp["bhh1"]).astype(f64)]

    def sig(x):
        return 1.0 / (1.0 + np.exp(-x))

    states = np.zeros((T, B, H), f64)
    for t in range(T):
        h = h * notdone[t][None, :, None]
        c = c * notdone[t][None, :, None]
        x = core_in[t]
        for l in (0, 1):
            g = x @ Wih[l].T + h[l] @ Whh[l].T + bl_[l]
            i_, f_, g_, o_ = np.split(g, 4, axis=-1)
            c[l] = sig(f_) * c[l] + sig(i_) * np.tanh(g_)
            x = sig(o_) * np.tanh(c[l])
            h[l] = x
        states[t] = h[1]

    at_tab = inp["actions_table"].astype(f64)
    ao_all = inp["actions_other"].reshape(S, AD).astype(f64)
    W_a0 = inp["W_a0"].astype(f64); b_a0 = inp["b_a0"].astype(f64)
    W_p0 = inp["W_p0"].astype(f64); b_p0 = inp["b_p0"].astype(f64)
    W_p1 = inp["W_p1"].astype(f64); b_p1 = inp["b_p1"].astype(f64)
    W_p2 = inp["W_p2"].astype(f64); b_p2 = inp["b_p2"].astype(f64)
    W_p3 = inp["W_p3"].astype(f64); b_p3 = inp["b_p3"].astype(f64)
    offs = inp["offset"].reshape(-1)
    cum = np.zeros(S + 1, np.int64)
    np.cumsum(offs, out=cum[1:])
    for t_, b_ in rows:
        tb = t_ * B + b_
        segs = tb * A + np.arange(A)
        at_rows = []
        for s_ in segs:
            rws = at_tab[cum[s_]:cum[s_ + 1]]
            ft = relu(relu(rws @ W_t0.T + b_t0) @ W_t1.T + b_t1)
            at_rows.append(ft.sum(axis=0))
        at_ = np.stack(at_rows)
        ao_ = relu(ao_all[segs] @ W_a0.T + b_a0)
        acts = np.concatenate([at_, ao_], axis=-1)
        srep = np.broadcast_to(states[t_, b_], (A, H))
        x = np.concatenate([srep, acts, srep * acts], axis=-1)
        x = relu(x @ W_p0.T + b_p0)
        x = relu(x @ W_p1.T + b_p1)
        x = relu(x @ W_p2.T + b_p2)
        lg = x @ W_p3.T + b_p3
        action[t_, b_] = int(np.argmax(lg[:, 0]))
    return action
